# revision 1
# baseline (speedup 1.0000x reference)
"""Trainium2 Bass kernel for nn_EyeRobotAgent block-sparse ("eye") attention.

Shapes: q,k,v [2, 12, 3456, 32] fp32.  S = 16 time-blocks x 216 feats.
Mask structure (per query block t):
  - all 216 keys of block t are candidates (minus img->img),
  - of each past block t-7..t-1, only 19 keys (m in {0..3, 5..19}) are
    visible (proprio m==4 and img m>=20 keys are never visible in the past),
  - joint queries (m in [4,20)) cannot see past joint keys,
  - img queries (m >= 20) cannot see img keys at all.

Strategy (data-parallel: 24 (b,h) pairs over 8 cores, 3 each):
  Pack per block t a compact key set [216 same | 133 past | 35 pad] = 384.
  The 2-D mask folds into the QK matmul via 3 extra contraction rows
  (rank-1 decomposition of the mask predicates); invalid/pad columns get a
  large negative bias so exp() underflows to 0.  Scores are computed
  transposed [kv, q] so probs can be consumed directly by the PV matmul,
  with a ones-column appended to V producing softmax denominators.
  exp() has no max-subtraction (scores are O(6), fp32-safe).
"""
import numpy as np

import concourse.bass as bass
import concourse.mybir as mybir
import concourse.tile as tile
from concourse import bacc
from concourse.bass_utils import run_bass_kernel_spmd
from concourse.masks import make_identity
from concourse.tile_rust import add_dep_helper

B, H, S, D = 2, 12, 3456, 32
F = 216            # feats_per_t
W = 8              # window_len
T = S // F         # 16 blocks
IMG_START = 20     # F - img_feat_size
JOINT_START = 4    # IMG_START - act_size
PAST_SEL = np.array([0, 1, 2, 3] + list(range(5, 20)))   # 19 per past block
NPAST = 19 * (W - 1)     # 133
KV = 384                 # packed kv per block (216 + 133 + pad)
KAUG = D + 3             # 35 contraction rows (32 d + 3 mask-bias rows)
VA = D + 1               # 33 = v columns + ones column
NEG = np.float32(-30000.0)
SCALE = float(1.0 / np.sqrt(np.float32(D)))
N_CORES = 8
BH_PER_CORE = (B * H) // N_CORES      # 3
NPAIR = T // 2                        # 8 block-pairs per (b,h)

F32 = mybir.dt.float32
BF16 = mybir.dt.float16      # half precision: matmul rate 1 cyc/row, 10-bit mantissa
NP_BF16 = np.float16


# ---------------------------------------------------------------- host packing

def _pack_all(q, k, v):
    """q,k,v: [B,H,S,D] fp32 ->
       qt  [24, KAUG, S]     (augmented Q^T)
       kpt [24, T, KAUG, KV] (augmented packed K^T per block)
       vp  [24, T, KV, VA]   (packed V + ones column per block)"""
    nbh = B * H
    qf = q.reshape(nbh, S, D)
    kf = k.reshape(nbh, S, D)
    vf = v.reshape(nbh, S, D)

    m = np.arange(F)
    is_img_m = (m >= IMG_START).astype(np.float32)
    is_joint_m = ((m >= JOINT_START) & (m < IMG_START)).astype(np.float32)
    qm = np.arange(S) % F

    qt = np.zeros((nbh, KAUG, S), np.float32)
    qt[:, :D] = qf.transpose(0, 2, 1)
    qt[:, 32] = (qm >= IMG_START)
    qt[:, 33] = (qm >= JOINT_START) & (qm < IMG_START)
    qt[:, 34] = 1.0

    kpt = np.zeros((nbh, T, KAUG, KV), np.float32)
    vp = np.zeros((nbh, T, KV, VA), np.float32)
    joint_past_bias = np.tile(NEG * is_joint_m[PAST_SEL], W - 1)  # [133]
    for t in range(T):
        blk = slice(F * t, F * (t + 1))
        kpt[:, t, :D, :F] = kf[:, blk].transpose(0, 2, 1)
        kpt[:, t, 32, :F] = NEG * is_img_m
        vp[:, t, :F, :D] = vf[:, blk]
        vp[:, t, :F, 32] = 1.0
        # past blocks t-7 .. t-1, 19 keys each
        taus = np.arange(t - 7, t)
        rows = (F * taus[:, None] + PAST_SEL[None, :]).reshape(-1)   # [133]
        valid = np.repeat(taus >= 0, 19)                             # [133]
        safe_rows = np.where(valid, rows, 0)
        pc = slice(F, F + NPAST)
        kpt[:, t, :D, pc] = np.where(
            valid[None, None, :], kf[:, safe_rows].transpose(0, 2, 1), 0.0)
        kpt[:, t, 33, pc] = joint_past_bias
        kpt[:, t, 34, pc] = np.where(valid, 0.0, NEG)
        vp[:, t, pc, :D] = np.where(
            valid[None, :, None], vf[:, safe_rows], 0.0)
        vp[:, t, pc, 32] = valid
        kpt[:, t, 34, F + NPAST:] = NEG        # pad columns
    # bulk per-bh DMA layouts:
    #   kpt2[bh, r, t, c]        = kpt[bh, t, r, c]
    #   vp2[bh, p, pair, c, tb*VA+n] = vp[bh, 2*pair+tb, 128*c+p, n]
    kpt2 = np.ascontiguousarray(kpt.transpose(0, 2, 1, 3))
    vp2 = vp.reshape(nbh, T // 2, 2, 3, 128, VA).transpose(0, 4, 1, 3, 2, 5)
    vp2 = np.ascontiguousarray(vp2.reshape(nbh, 128, T // 2, 3, 2 * VA))
    return (qt.astype(NP_BF16), kpt2.astype(NP_BF16), vp2.astype(NP_BF16))


# ---------------------------------------------------------------- bass kernel

def build_nc(n_bh=BH_PER_CORE, n_pairs=NPAIR):
    nc = bacc.Bacc(None, target_bir_lowering=False, debug=False)
    qt_d = nc.declare_dram_parameter("qt", [BH_PER_CORE, KAUG, S], BF16, isOutput=False)
    kpt_d = nc.declare_dram_parameter("kpt", [BH_PER_CORE, KAUG, T, KV], BF16, isOutput=False)
    vp_d = nc.declare_dram_parameter("vp", [BH_PER_CORE, 128, T // 2, 3, 2 * VA], BF16, isOutput=False)
    out_d = nc.declare_dram_parameter("out", [BH_PER_CORE, S, D], F32, isOutput=True)

    def _strided2(ap, d1, d2):
        return bass.AP(tensor=ap.tensor, offset=ap.offset,
                       ap=[list(ap.ap[0]), list(d1), list(d2)])

    with tile.TileContext(nc) as tc:
        with (
            tc.tile_pool(name="singles", bufs=1) as singles,
            tc.tile_pool(name="qtp", bufs=3) as qtp,
            tc.tile_pool(name="kptp", bufs=3) as kptp,
            tc.tile_pool(name="vpp", bufs=3) as vpp,
            tc.tile_pool(name="probsp", bufs=3) as probsp,
            tc.tile_pool(name="pvsbp", bufs=3) as pvsbp,
            tc.tile_pool(name="recipsp", bufs=3) as recipsp,
            tc.tile_pool(name="outsbp", bufs=3) as outsbp,
            tc.tile_pool(name="scoresp", bufs=2, space="PSUM") as scoresp,
            tc.tile_pool(name="pvp", bufs=1, space="PSUM") as pvp,
        ):
            ident = singles.tile([128, 128], F32)
            make_identity(nc, ident[:])

            for i in range(n_bh):
                qt_sb = qtp.tile([KAUG, S], BF16)
                kpt_sb = kptp.tile([KAUG, T, KV], BF16)
                vp_sb = vpp.tile([128, T // 2, 3, 2 * VA], BF16)
                for hf in range(2):
                    hs, ts_, ps_ = S // 2 * hf, T // 2 * hf, NPAIR // 2 * hf
                    nc.sync.dma_start(out=qt_sb[:, hs:hs + S // 2],
                                      in_=qt_d[i, :, hs:hs + S // 2])
                    nc.sync.dma_start(out=kpt_sb[:, ts_:ts_ + T // 2, :],
                                      in_=kpt_d[i, :, ts_:ts_ + T // 2, :])
                    nc.sync.dma_start(
                        out=vp_sb[:, ps_:ps_ + NPAIR // 2, :, :],
                        in_=vp_d[i, :, ps_:ps_ + NPAIR // 2, :, :])
                outst = outsbp.tile([128, NPAIR * 128], F32)


                for p in range(n_pairs):
                    t0 = 2 * p

                    # ---- QK^T (transposed scores [kv, q]), mask via bias rows
                    scores = scoresp.tile([128, 1536], F32)   # 3 psum banks
                    for c in range(3):
                        first = None
                        for tb in range(2):
                            mm = nc.tensor.matmul(
                                scores[:, 512 * c + 216 * tb:512 * c + 216 * tb + 216],
                                lhsT=kpt_sb[:, t0 + tb, 128 * c:128 * c + 128],
                                rhs=qt_sb[:, 216 * (t0 + tb):216 * (t0 + tb) + 216],
                                start=(tb == 0), stop=(tb == 1))
                            if tb == 0:
                                first = mm
                            else:
                                add_dep_helper(mm.ins, first.ins, sync=False,
                                               reason="qk same-bank group order")

                    # ---- probs = exp(scale * scores), one ACT op
                    probs = probsp.tile([128, 1296], BF16)
                    sc_v = scores[:].rearrange("p (c x) -> p c x", c=3)[:, :, 0:432]
                    pr_v = probs[:].rearrange("p (c x) -> p c x", c=3)
                    nc.scalar.activation(pr_v, sc_v,
                                         mybir.ActivationFunctionType.Exp,
                                         scale=SCALE)

                    # ---- PV: out_T[va, q]; one psum bank per block
                    # (separate banks avoid interleaved-group pending-zero
                    # hazards and cross-engine bank overlap).
                    pv = pvp.tile([128, 1024], F32)
                    for tb in range(2):
                        for c in range(3):
                            nc.tensor.matmul(
                                pv[0:VA, 512 * tb:512 * tb + 216],
                                lhsT=vp_sb[:, p, c, VA * tb:VA * tb + VA],
                                rhs=probs[:, 432 * c + 216 * tb:432 * c + 216 * tb + 216],
                                start=(c == 0), stop=(c == 2))

                    # ---- evacuate both blocks to SBUF in one DVE op
                    pvsb = pvsbp.tile([VA, 2, 216], F32)
                    cp1 = nc.vector.tensor_copy(
                        pvsb[:],
                        _strided2(pv[0:VA, 0:1], (512, 2), (1, 216)))

                    # ---- PE-transpose each q-slice into the scores tile's
                    # spare columns (exp already consumed those banks; this
                    # frees the pv tile for the next pair right after cp1).
                    # qs=0 slots (128-wide) -> bank0 col 432+33*tb;
                    # qs=1 slots ( 88-wide) -> bank1 col 944+33*tb.
                    prevs = [None, None]
                    for tb in range(2):
                        for qs in range(2):
                            w = 128 if qs == 0 else 88
                            col = (432 if qs == 0 else 944) + 33 * tb
                            mm = nc.tensor.matmul(
                                scores[0:w, col:col + VA],
                                lhsT=pvsb[:, tb, 128 * qs:128 * qs + w],
                                rhs=ident[0:VA, 0:VA],
                                is_transpose=True,
                                start=(tb == 0), stop=(tb == 1))
                            if prevs[qs] is not None:
                                add_dep_helper(mm.ins, prevs[qs].ins,
                                               sync=False,
                                               reason="tr bank group order")
                            prevs[qs] = mm

                    # ---- normalize: out = num * (1/den), split by q-width
                    # (all reader->writer hazards here are RAW-tracked: each
                    # recip/mul reads both of its bank's transpose outputs)
                    recips = recipsp.tile([128, 4], F32)
                    nc.vector.reciprocal(recips[0:128, 0:3:2],
                                         scores[0:128, 464:498:33])
                    nc.vector.reciprocal(recips[0:88, 1:4:2],
                                         scores[0:88, 976:1010:33])
                    _strided = _strided2

                    for qs, w in ((0, 128), (1, 88)):
                        col = 432 if qs == 0 else 944
                        num_v = _strided(scores[0:w, col:col + 1],
                                         (33, 2), (1, 32))
                        rec_bcast = _strided(recips[0:w, qs:qs + 1],
                                             (2, 2), (0, 32))
                        out_v = _strided(
                            outst[0:w, 128 * p + 32 * qs:128 * p + 32 * qs + 1],
                            (64, 2), (1, 32))
                        nc.vector.tensor_mul(out_v, num_v, rec_bcast)

                    # ---- store half-(b,h) after pairs 0-3 / 4-7 complete
                    if p % (NPAIR // 2) == NPAIR // 2 - 1:
                        hf = p // (NPAIR // 2)
                        po = NPAIR // 2 * hf
                        for qs, w in ((0, 128), (1, 88)):
                            dst = bass.AP(
                                tensor=out_d,
                                offset=(i * S + 432 * po + 128 * qs) * D,
                                ap=[[D, w], [432 * D, NPAIR // 2],
                                    [216 * D, 2], [1, D]])
                            sap = outst[:]
                            sst = bass.AP(
                                tensor=sap.tensor,
                                offset=sap.offset + 128 * po + 32 * qs,
                                ap=[[sap.ap[0][0], w], [128, NPAIR // 2],
                                    [64, 2], [1, 32]])
                            nc.sync.dma_start(out=dst, in_=sst)
    nc.compile()
    return nc


_NC = None


def _get_nc():
    global _NC
    if _NC is None:
        _NC = build_nc()
    return _NC


# ---------------------------------------------------------------- entry point

def kernel(q, k, v, feats_per_t, window_len, act_size, img_feat_size):
    assert int(feats_per_t) == F and int(window_len) == W
    assert int(act_size) == 16 and int(img_feat_size) == 196
    q = np.asarray(q, np.float32)
    k = np.asarray(k, np.float32)
    v = np.asarray(v, np.float32)

    qt, kpt, vp = _pack_all(q, k, v)
    in_maps = []
    for core in range(N_CORES):
        s = slice(BH_PER_CORE * core, BH_PER_CORE * (core + 1))
        in_maps.append({"qt": np.ascontiguousarray(qt[s]),
                        "kpt": np.ascontiguousarray(kpt[s]),
                        "vp": np.ascontiguousarray(vp[s])})

    nc = _get_nc()
    res = run_bass_kernel_spmd(nc, in_maps, list(range(N_CORES)))
    out = np.empty((B * H, S, D), np.float32)
    for core in range(N_CORES):
        out[BH_PER_CORE * core:BH_PER_CORE * (core + 1)] = res.results[core]["out"]
    return out.reshape(B, H, S, D)



# revision 6
# speedup vs baseline: 1.5731x; 1.5731x over previous
"""Trainium2 Bass kernel for nn_EyeRobotAgent block-sparse ("eye") attention.

Shapes: q,k,v [2, 12, 3456, 32] fp32.  S = 16 time-blocks x 216 feats.
Mask structure (per query block t):
  - all 216 keys of block t are candidates (minus img->img),
  - of each past block t-7..t-1, only 19 keys (m in {0..3, 5..19}) are
    visible (proprio m==4 and img m>=20 keys are never visible in the past),
  - joint queries (m in [4,20)) cannot see past joint keys,
  - img queries (m >= 20) cannot see img keys at all.

Strategy (data-parallel: 24 (b,h) pairs over 8 cores, 3 each):
  Pack per block t a compact key set [216 same | 133 past | 35 pad] = 384.
  The 2-D mask folds into the QK matmul via 3 extra contraction rows
  (rank-1 decomposition of the mask predicates); invalid/pad columns get a
  large negative bias so exp() underflows to 0.  Scores are computed
  transposed [kv, q] so probs can be consumed directly by the PV matmul,
  with a ones-column appended to V producing softmax denominators.
  exp() has no max-subtraction (scores are O(6), fp32-safe).
"""
import numpy as np

import concourse.bass as bass
import concourse.mybir as mybir
import concourse.tile as tile
from concourse import bacc
from concourse.bass_utils import run_bass_kernel_spmd
from concourse.masks import make_identity
from concourse.tile_rust import add_dep_helper

B, H, S, D = 2, 12, 3456, 32
F = 216            # feats_per_t
W = 8              # window_len
T = S // F         # 16 blocks
IMG_START = 20     # F - img_feat_size
JOINT_START = 4    # IMG_START - act_size
PAST_SEL = np.array([0, 1, 2, 3] + list(range(5, 20)))   # 19 per past block
NPAST = 19 * (W - 1)     # 133
KV = 384                 # packed kv per block (216 + 133 + pad)
KAUG = D + 3             # 35 contraction rows (32 d + 3 mask-bias rows)
VA = D + 1               # 33 = v columns + ones column
NEG = np.float32(-30000.0)
SCALE = float(1.0 / np.sqrt(np.float32(D)))
N_CORES = 8
BH_PER_CORE = (B * H) // N_CORES      # 3
NPAIR = T // 2                        # 8 block-pairs per (b,h)

F32 = mybir.dt.float32
BF16 = mybir.dt.float16      # half precision: matmul rate 1 cyc/row, 10-bit mantissa
NP_BF16 = np.float16


# ---------------------------------------------------------------- host packing

def _pack_all(q, k, v):
    """q,k,v: [B,H,S,D] fp32 ->
       qt  [24, KAUG, S]     (augmented Q^T)
       kpt [24, T, KAUG, KV] (augmented packed K^T per block)
       vp  [24, T, KV, VA]   (packed V + ones column per block)"""
    nbh = B * H
    qf = q.reshape(nbh, S, D)
    kf = k.reshape(nbh, S, D)
    vf = v.reshape(nbh, S, D)

    m = np.arange(F)
    is_img_m = (m >= IMG_START).astype(np.float32)
    is_joint_m = ((m >= JOINT_START) & (m < IMG_START)).astype(np.float32)
    qm = np.arange(S) % F

    qt = np.zeros((nbh, KAUG, S), np.float32)
    qt[:, :D] = qf.transpose(0, 2, 1)
    qt[:, 32] = (qm >= IMG_START)
    qt[:, 33] = (qm >= JOINT_START) & (qm < IMG_START)
    qt[:, 34] = 1.0

    kpt = np.zeros((nbh, T, KAUG, KV), np.float32)
    vp = np.zeros((nbh, T, KV, VA), np.float32)
    joint_past_bias = np.tile(NEG * is_joint_m[PAST_SEL], W - 1)  # [133]
    for t in range(T):
        blk = slice(F * t, F * (t + 1))
        kpt[:, t, :D, :F] = kf[:, blk].transpose(0, 2, 1)
        kpt[:, t, 32, :F] = NEG * is_img_m
        vp[:, t, :F, :D] = vf[:, blk]
        vp[:, t, :F, 32] = 1.0
        # past blocks t-7 .. t-1, 19 keys each
        taus = np.arange(t - 7, t)
        rows = (F * taus[:, None] + PAST_SEL[None, :]).reshape(-1)   # [133]
        valid = np.repeat(taus >= 0, 19)                             # [133]
        safe_rows = np.where(valid, rows, 0)
        pc = slice(F, F + NPAST)
        kpt[:, t, :D, pc] = np.where(
            valid[None, None, :], kf[:, safe_rows].transpose(0, 2, 1), 0.0)
        kpt[:, t, 33, pc] = joint_past_bias
        kpt[:, t, 34, pc] = np.where(valid, 0.0, NEG)
        vp[:, t, pc, :D] = np.where(
            valid[None, :, None], vf[:, safe_rows], 0.0)
        vp[:, t, pc, 32] = valid
        kpt[:, t, 34, F + NPAST:] = NEG        # pad columns
    # bulk per-bh DMA layouts:
    #   kpt2[bh, r, t, c]        = kpt[bh, t, r, c]
    #   vp2[bh, p, pair, c, tb*VA+n] = vp[bh, 2*pair+tb, 128*c+p, n]
    kpt2 = np.ascontiguousarray(kpt.transpose(0, 2, 1, 3))
    vp2 = vp.reshape(nbh, T // 2, 2, 3, 128, VA).transpose(0, 4, 1, 3, 2, 5)
    vp2 = np.ascontiguousarray(vp2.reshape(nbh, 128, T // 2, 3, 2 * VA))
    return (qt.astype(NP_BF16), kpt2.astype(NP_BF16), vp2.astype(NP_BF16))


# ---------------------------------------------------------------- bass kernel

def build_nc(n_bh=BH_PER_CORE, n_pairs=NPAIR):
    nc = bacc.Bacc(None, target_bir_lowering=False, debug=False)
    qt_d = nc.declare_dram_parameter("qt", [BH_PER_CORE, KAUG, S], BF16, isOutput=False)
    kpt_d = nc.declare_dram_parameter("kpt", [BH_PER_CORE, KAUG, T, KV], BF16, isOutput=False)
    vp_d = nc.declare_dram_parameter("vp", [BH_PER_CORE, 128, T // 2, 3, 2 * VA], BF16, isOutput=False)
    # out layout mirrors the SBUF staging tile exactly (big contiguous DMA
    # runs); host unpacks.  col = 128*pair + 64*tb + 32*qs + d,
    # partition = q - 216*block - 128*qs.
    out_d = nc.declare_dram_parameter("out", [BH_PER_CORE, 128, NPAIR * 128], F32, isOutput=True)

    def _strided2(ap, d1, d2):
        return bass.AP(tensor=ap.tensor, offset=ap.offset,
                       ap=[list(ap.ap[0]), list(d1), list(d2)])

    with tile.TileContext(nc) as tc:
        with (
            tc.tile_pool(name="qtp", bufs=3) as qtp,
            tc.tile_pool(name="kptp", bufs=3) as kptp,
            tc.tile_pool(name="vpp", bufs=3) as vpp,
            tc.tile_pool(name="probsp", bufs=3) as probsp,
            tc.tile_pool(name="recipsp", bufs=3) as recipsp,
            tc.tile_pool(name="outsbp", bufs=3) as outsbp,
            tc.tile_pool(name="scoresp", bufs=2, space="PSUM") as scoresp,
            tc.tile_pool(name="pvp", bufs=2, space="PSUM") as pvp,
        ):
            for i in range(n_bh):
                qt_sb = qtp.tile([KAUG, S], BF16)
                kpt_sb = kptp.tile([KAUG, T, KV], BF16)
                vp_sb = vpp.tile([128, T // 2, 3, 2 * VA], BF16)
                for hf in range(2):
                    hs, ts_, ps_ = S // 2 * hf, T // 2 * hf, NPAIR // 2 * hf
                    nc.sync.dma_start(out=qt_sb[:, hs:hs + S // 2],
                                      in_=qt_d[i, :, hs:hs + S // 2])
                    nc.sync.dma_start(out=kpt_sb[:, ts_:ts_ + T // 2, :],
                                      in_=kpt_d[i, :, ts_:ts_ + T // 2, :])
                    nc.sync.dma_start(
                        out=vp_sb[:, ps_:ps_ + NPAIR // 2, :, :],
                        in_=vp_d[i, :, ps_:ps_ + NPAIR // 2, :, :])
                outst = outsbp.tile([128, NPAIR * 128], F32)


                for p in range(n_pairs):
                    t0 = 2 * p

                    # ---- QK^T (transposed scores [kv, q]), mask via bias rows
                    scores = scoresp.tile([128, 1536], F32)   # 3 psum banks
                    for c in range(3):
                        first = None
                        for tb in range(2):
                            mm = nc.tensor.matmul(
                                scores[:, 512 * c + 216 * tb:512 * c + 216 * tb + 216],
                                lhsT=kpt_sb[:, t0 + tb, 128 * c:128 * c + 128],
                                rhs=qt_sb[:, 216 * (t0 + tb):216 * (t0 + tb) + 216],
                                start=(tb == 0), stop=(tb == 1))
                            if tb == 0:
                                first = mm
                            else:
                                add_dep_helper(mm.ins, first.ins, sync=False,
                                               reason="qk same-bank group order")

                    # ---- probs = exp(scale * scores), one ACT op
                    probs = probsp.tile([128, 1296], BF16)
                    sc_v = scores[:].rearrange("p (c x) -> p c x", c=3)[:, :, 0:432]
                    pr_v = probs[:].rearrange("p (c x) -> p c x", c=3)
                    nc.scalar.activation(pr_v, sc_v,
                                         mybir.ActivationFunctionType.Exp,
                                         scale=SCALE)

                    # ---- PV flipped: probs is the stationary operand, so
                    # each matmul streams only VA=33 columns.  out[q, va]
                    # directly (no transpose needed); denominator in col 32.
                    # 4 sequential accumulation groups in one psum bank:
                    # col = 33*(2*tb+qs).
                    pv = pvp.tile([128, 132], F32)
                    prev = None
                    for tb in range(2):
                        for qs, w in ((0, 128), (1, 88)):
                            g = 33 * (2 * tb + qs)
                            for c in range(3):
                                mm = nc.tensor.matmul(
                                    pv[0:w, g:g + VA],
                                    lhsT=probs[:, 432 * c + 216 * tb + 128 * qs:
                                               432 * c + 216 * tb + 128 * qs + w],
                                    rhs=vp_sb[:, p, c, VA * tb:VA * tb + VA],
                                    start=(c == 0), stop=(c == 2))
                                if prev is not None:
                                    add_dep_helper(mm.ins, prev.ins, sync=False,
                                                   reason="pv same-bank group order")
                                prev = mm

                    # ---- normalize: out = num * (1/den), split by q-width
                    recips = recipsp.tile([128, 4], F32)
                    nc.vector.reciprocal(recips[0:128, 0:3:2],
                                         pv[0:128, 32:132:66])
                    nc.vector.reciprocal(recips[0:88, 1:4:2],
                                         pv[0:88, 65:132:66])

                    for qs, w in ((0, 128), (1, 88)):
                        num_v = _strided2(pv[0:w, 33 * qs:33 * qs + 1],
                                          (66, 2), (1, 32))
                        rec_bcast = _strided2(recips[0:w, qs:qs + 1],
                                              (2, 2), (0, 32))
                        out_v = _strided2(
                            outst[0:w, 128 * p + 32 * qs:128 * p + 32 * qs + 1],
                            (64, 2), (1, 32))
                        nc.vector.tensor_mul(out_v, num_v, rec_bcast)

                    # ---- store half-(b,h) after pairs 0-3 / 4-7 complete
                    # (partitions 88..127 of qs1 col-groups are never written
                    # -> transfer rows 0:88 fully, rows 88:128 qs0-cols only)
                    if p % (NPAIR // 2) == NPAIR // 2 - 1:
                        hf = p // (NPAIR // 2)
                        nc.sync.dma_start(
                            out=out_d[i, 0:88, 512 * hf:512 * hf + 512],
                            in_=outst[0:88, 512 * hf:512 * hf + 512])
                        sap = outst[:]
                        src = bass.AP(
                            tensor=sap.tensor,
                            offset=sap.offset + 88 * sap.ap[0][0] + 512 * hf,
                            ap=[[sap.ap[0][0], 40], [128, 4], [64, 2], [1, 32]])
                        dst = bass.AP(
                            tensor=out_d,
                            offset=(i * 128 + 88) * (NPAIR * 128) + 512 * hf,
                            ap=[[NPAIR * 128, 40], [128, 4], [64, 2], [1, 32]])
                        nc.sync.dma_start(out=dst, in_=src)
    nc.compile()
    return nc


_NC = None


def _get_nc():
    global _NC
    if _NC is None:
        _NC = build_nc()
    return _NC


# ---------------------------------------------------------------- entry point

def kernel(q, k, v, feats_per_t, window_len, act_size, img_feat_size):
    assert int(feats_per_t) == F and int(window_len) == W
    assert int(act_size) == 16 and int(img_feat_size) == 196
    q = np.asarray(q, np.float32)
    k = np.asarray(k, np.float32)
    v = np.asarray(v, np.float32)

    qt, kpt, vp = _pack_all(q, k, v)
    in_maps = []
    for core in range(N_CORES):
        s = slice(BH_PER_CORE * core, BH_PER_CORE * (core + 1))
        in_maps.append({"qt": np.ascontiguousarray(qt[s]),
                        "kpt": np.ascontiguousarray(kpt[s]),
                        "vp": np.ascontiguousarray(vp[s])})

    nc = _get_nc()
    res = run_bass_kernel_spmd(nc, in_maps, list(range(N_CORES)))
    out = np.empty((B * H, S, D), np.float32)
    for core in range(N_CORES):
        arr = res.results[core]["out"]          # [3, 128, NPAIR*128]
        a = arr.reshape(BH_PER_CORE, 128, NPAIR, 2, 2, 32)
        a = a.transpose(0, 2, 3, 4, 1, 5).reshape(BH_PER_CORE, T, 256, 32)
        out[BH_PER_CORE * core:BH_PER_CORE * (core + 1)] = (
            a[:, :, :F].reshape(BH_PER_CORE, S, D))
    return out.reshape(B, H, S, D)



# revision 24
# speedup vs baseline: 1.6223x; 1.0313x over previous
"""Trainium2 Bass kernel for nn_EyeRobotAgent block-sparse ("eye") attention.

Shapes: q,k,v [2, 12, 3456, 32] fp32.  S = 16 time-blocks x 216 feats.
Mask structure (per query block t):
  - img queries (m in [20,216), 196 of them) see only the "core" keys:
    19 keys (m in {0..3,5..19}) of each block tau in [t-7, t] plus m4(t)
    -> at most 153 keys,
  - non-img queries (m in [0,20), 20 of them) see core keys + the 196 img
    keys of block t (joint queries additionally lose past joint keys,
    handled by a bias row).

Strategy (data-parallel: 24 (b,h) pairs over 8 cores, 3 each):
  Per block pack keys as [core 153 | img 196 | pad] = 384 (newest-first so
  invalid tail cols are contiguous; masks fold into 2 bias contraction
  rows).  Scores are computed transposed [kv, q] in per-quad (4 block)
  PSUM tiles so a single ACT exp covers ~1000-1250 columns:
    N_j: 20 non-img queries vs 2-3 full-height 128-row chunks of the pack
    A_j: 196 img queries vs core[0:128]
    C:   core[128:153] tails of the 4 blocks packed into 32-row PE
         quadrant bands (tile_position rows 32j), one shared 196-col region
    PAD: 28 dummy cols kept defined so PV lhsT "spill" reads stay legal.
  PV uses probs as the stationary operand (out[q, 33] per matmul streams
  only 33 columns); the appended ones-column of V yields the softmax
  denominator in col 32; normalize is one DVE reciprocal + one mul per
  quad, padded out-groups making every partition defined.
  exp() has no max-subtraction (scores are O(6), fp32-safe).
"""
import math
import numpy as np

import concourse.bass as bass
import concourse.mybir as mybir
import concourse.tile as tile
from concourse import bacc
from concourse.bass_utils import run_bass_kernel_spmd
from concourse.tile_rust import add_dep_helper

B, H, S, D = 2, 12, 3456, 32
F = 216            # feats_per_t
W = 8              # window_len
T = S // F         # 16 blocks
IMG_START = 20     # F - img_feat_size
PAST_SEL = np.array([0, 1, 2, 3] + list(range(5, 20)))   # 19 per block
NCORE = 153        # 8*19 + 1 (m4) candidate core keys
NIMG = 196
PACK = 384         # [core (<=153, compact) | img 196 | pad]
KAUG = D + 3       # 35 = 32 d + joint-bias + img-img bias + validity rows
VA = D + 1         # 33 = v columns + ones column
NEG = np.float32(-30000.0)
SCALE = float(1.0 / np.sqrt(np.float32(D)))
N_CORES = 8
BH_PER_CORE = (B * H) // N_CORES      # 3
NQ = 4                                # blocks per quad
QUADS = T // NQ                       # 4

F32 = mybir.dt.float32
BF16 = mybir.dt.float16      # half precision: matmul rate 1 cyc/row
NP_BF16 = np.float16


def _nvalid_core(t):
    return 20 + 19 * min(t, 7)


def _n_chunks(t):
    return math.ceil((_nvalid_core(t) + NIMG) / 128)


def _pack_rows(t):
    """Compact key packing for block t: [19(t), m4(t), 19(t-1), ..,
    19(t-min(t,7)), img(t) 196, pad].  -1 marks invalid (trailing only)."""
    rows = list(F * t + PAST_SEL) + [F * t + 4]
    for s in range(1, min(t, 7) + 1):
        rows += list(F * (t - s) + PAST_SEL)
    rows += list(range(F * t + IMG_START, F * t + F))
    rows += [-1] * (PACK - len(rows))
    return np.array(rows)


def _quad_layout(g):
    """Column layout of the per-quad score tile, bank-aware (matmul outputs
    must not cross 512-col PSUM bank boundaries): bank0 = [A0 A1 N x6],
    bank1 = [A2 A3 N-rest pad?], bank2 = [C... pad].  C regions are per
    block PAIR (bands rows 0:32 / 32:64 by t%2, rows 64:128 dummy-filled).
    Exp'd cols [0:ncols] are gap-free; 60 pad cols keep PV spill reads
    defined."""
    ts = list(range(NQ * g, NQ * g + NQ))
    chunks = [(j, c) for j, t in enumerate(ts) for c in range(_n_chunks(t))]
    a_off = [0, 196, 512, 708]
    n_cols = {}
    col = 392
    for jc in chunks[:6]:
        n_cols[jc] = col
        col += 20
    assert col <= 512
    col = 904
    for jc in chunks[6:]:
        n_cols[jc] = col
        col += 20
    pads = []
    if col < 1024:
        pads.append((col, 1024 - col))
    c_off = {}
    cbase = 1024
    for pl in range(NQ // 2):           # local pair index
        if any(_nvalid_core(t) > 128 for t in ts[2 * pl:2 * pl + 2]):
            c_off[pl] = cbase
            cbase += NIMG
    pads.append((cbase, 60))
    return ts, n_cols, a_off, c_off, pads, cbase + 60


# ---------------------------------------------------------------- host packing

def _pack_all(q, k, v):
    nbh = B * H
    qf = q.reshape(nbh, S, D)
    kf = k.reshape(nbh, S, D)
    vf = v.reshape(nbh, S, D)
    qm = np.arange(S) % F

    qt = np.zeros((nbh, KAUG, S), np.float32)
    qt[:, :D] = qf.transpose(0, 2, 1)
    qt[:, 32] = (qm >= 4) & (qm < IMG_START)      # is_joint(q)
    qt[:, 33] = qm >= IMG_START                   # is_img(q)
    qt[:, 34] = 1.0

    kpt = np.zeros((nbh, KAUG, T, PACK), np.float32)
    vp = np.zeros((nbh, 128, T, 3, VA), np.float32)
    vpc = np.zeros((nbh, 128, T // 2, VA), np.float32)
    for t in range(T):
        rows = _pack_rows(t)
        valid = rows >= 0
        safe = np.where(valid, rows, 0)
        kpt[:, :D, t] = np.where(valid[None, None], kf[:, safe].transpose(0, 2, 1), 0.0)
        # joint-past bias: past sets s=1..min(t,7) at cols 20+19(s-1),
        # joint keys at positions 4..18 within each set
        jbias = np.zeros(PACK, np.float32)
        for s in range(1, min(t, 7) + 1):
            base = 20 + 19 * (s - 1)
            jbias[base + 4: base + 19] = NEG
        kpt[:, 32, t] = jbias
        kpt[:, 33, t] = np.where(valid & (rows % F >= IMG_START), NEG, 0.0)
        kpt[:, 34, t] = np.where(valid, 0.0, NEG)
        vblk = np.where(valid[None, :, None], vf[:, safe], 0.0)   # [nbh,384,32]
        vp[:, :, t, :, :D] = vblk.reshape(nbh, 3, 128, D).transpose(0, 2, 1, 3)
        # ones column: eps (not 0) on invalid rows keeps every PV spill-row
        # denominator strictly positive (invalid probs are exactly 0, so
        # valid outputs are unaffected).
        ones = np.where(valid, 1.0, 6e-5)
        vp[:, :, t, :, 32] = ones.reshape(3, 128).T[None]
        pr, b = t // 2, 32 * (t % 2)
        # C tail: only core positions 128..nvalid_core-1 (img keys that
        # fall in [128:153] of the compact pack must contribute zero)
        ncv = _nvalid_core(t)
        tail = safe[128:NCORE]
        tval = np.arange(128, NCORE) < ncv
        vpc[:, b:b + 25, pr, :D] = np.where(tval[None, :, None], vf[:, tail], 0.0)
        vpc[:, b:b + 25, pr, 32] = np.where(tval, 1.0, 0.0)
    return {"qt": qt.astype(NP_BF16),
            "kpt": np.ascontiguousarray(kpt.astype(NP_BF16)),
            "vp": np.ascontiguousarray(vp.astype(NP_BF16)),
            "vpc": np.ascontiguousarray(vpc.astype(NP_BF16))}


def _unpack(arr):
    """arr [n, 128, QUADS*320] staging -> [n, S, D].  Per quad g, 10 groups
    of 32 cols: j=0..3 img q 20..147 (rows 0:128); 4..7 img q 148..215
    (rows 0:68); 8..9 non-img q 0..19 of blocks 2p (rows 0:20) and 2p+1
    (rows 64:84)."""
    n = arr.shape[0]
    r = arr.reshape(n, 128, QUADS, 10, 32).transpose(0, 2, 3, 1, 4)
    out = np.empty((n, QUADS, NQ, F, D), np.float32)
    for j in range(NQ):
        out[:, :, j, IMG_START:148] = r[:, :, j, 0:128]
        out[:, :, j, 148:] = r[:, :, 4 + j, 0:68]
        out[:, :, j, :IMG_START] = r[:, :, 8 + j // 2, 64 * (j % 2):64 * (j % 2) + 20]
    return out.reshape(n, S, D)


# ---------------------------------------------------------------- bass kernel

def build_nc(n_bh=BH_PER_CORE):
    nc = bacc.Bacc(None, target_bir_lowering=False, debug=False)
    qt_d = nc.declare_dram_parameter("qt", [n_bh, KAUG, S], BF16, isOutput=False)
    kpt_d = nc.declare_dram_parameter("kpt", [n_bh, KAUG, T, PACK], BF16, isOutput=False)
    vp_d = nc.declare_dram_parameter("vp", [n_bh, 128, T, 3, VA], BF16, isOutput=False)
    vpc_d = nc.declare_dram_parameter("vpc", [n_bh, 128, T // 2, VA], BF16, isOutput=False)
    # out mirrors the SBUF staging tile exactly; host unpacks.
    out_d = nc.declare_dram_parameter("out", [n_bh, 128, QUADS * 320], F32, isOutput=True)

    def _strided2(ap, d1, d2):
        return bass.AP(tensor=ap.tensor, offset=ap.offset,
                       ap=[list(ap.ap[0]), list(d1), list(d2)])

    with tile.TileContext(nc) as tc:
        with (
            tc.tile_pool(name="singles", bufs=1) as singles,
            tc.tile_pool(name="qtp", bufs=3) as qtp,
            tc.tile_pool(name="kptp", bufs=3) as kptp,
            tc.tile_pool(name="vpp", bufs=3) as vpp,
            tc.tile_pool(name="vpcp", bufs=3) as vpcp,
            tc.tile_pool(name="probsp", bufs=3) as probsp,
            tc.tile_pool(name="recipsp", bufs=3) as recipsp,
            tc.tile_pool(name="outsbp", bufs=3) as outsbp,
            tc.tile_pool(name="scoresp", bufs=2, space="PSUM") as scoresp,
            tc.tile_pool(name="pvp", bufs=2, space="PSUM") as pvp,
        ):
            zero = singles.tile([1, 128], BF16)
            nc.vector.memset(zero[:], 0.0)

            for i in range(n_bh):
                qt_sb = qtp.tile([KAUG, S], BF16)
                kpt_sb = kptp.tile([KAUG, T, PACK], BF16)
                vp_sb = vpp.tile([128, T, 3, VA], BF16)
                vpc_sb = vpcp.tile([128, T // 2, VA], BF16)
                for hf in range(2):
                    hs, ts_ = S // 2 * hf, T // 2 * hf
                    nc.sync.dma_start(out=qt_sb[:, hs:hs + S // 2],
                                      in_=qt_d[i, :, hs:hs + S // 2])
                    nc.sync.dma_start(out=kpt_sb[:, ts_:ts_ + T // 2, :],
                                      in_=kpt_d[i, :, ts_:ts_ + T // 2, :])
                    nc.sync.dma_start(out=vp_sb[:, ts_:ts_ + T // 2, :, :],
                                      in_=vp_d[i, :, ts_:ts_ + T // 2, :, :])
                nc.sync.dma_start(out=vpc_sb[:], in_=vpc_d[i])
                outst = outsbp.tile([128, QUADS * 320], F32)

                for g in range(QUADS):
                    ts, n_cols, a_off, c_off, pads, ncols = _quad_layout(g)

                    # ---- QK^T (transposed scores [kv, q]); masks via the 2
                    # bias rows; all mms chained (same-bank group ordering).
                    scores = scoresp.tile([128, 1536], F32)
                    prev = None

                    def mm(out, lhsT, rhs, start=True, stop=True):
                        nonlocal prev
                        m = nc.tensor.matmul(out, lhsT=lhsT, rhs=rhs,
                                             start=start, stop=stop)
                        if prev is not None:
                            add_dep_helper(m.ins, prev.ins, sync=False,
                                           reason="bank group order")
                        prev = m
                        return m

                    for j, t in enumerate(ts):
                        for c in range(_n_chunks(t)):
                            nco = n_cols[(j, c)]
                            mm(scores[0:128, nco:nco + 20],
                               kpt_sb[:, t, 128 * c:128 * c + 128],
                               qt_sb[:, F * t:F * t + 20])
                    for j, t in enumerate(ts):
                        mm(scores[0:128, a_off[j]:a_off[j] + NIMG],
                           kpt_sb[:, t, 0:128],
                           qt_sb[:, F * t + IMG_START:F * t + F])
                    for pl, co in c_off.items():
                        for t in ts[2 * pl:2 * pl + 2]:
                            b = 32 * (t % 2)
                            if _nvalid_core(t) > 128:
                                mm(scores[b:b + 32, co:co + NIMG],
                                   kpt_sb[:, t, 128:160],
                                   qt_sb[:, F * t + IMG_START:F * t + F])
                            else:      # dummy fill: zero scores -> probs 1
                                mm(scores[b:b + 32, co:co + NIMG],
                                   zero[0:1, 0:32], qt_sb[0:1, 0:NIMG])
                        mm(scores[64:128, co:co + NIMG],
                           zero[0:1, 0:64], qt_sb[0:1, 0:NIMG])
                    for poff, pw in pads:
                        mm(scores[0:128, poff:poff + pw],
                           zero[0:1, 0:128], qt_sb[0:1, 0:pw])

                    # ---- probs = exp(scale * scores), one ACT op per quad
                    probs = probsp.tile([128, 1536], BF16)
                    nc.scalar.activation(probs[0:128, 0:ncols],
                                         scores[0:128, 0:ncols],
                                         mybir.ActivationFunctionType.Exp,
                                         scale=SCALE)

                    # ---- PV: probs stationary, out[q, va] (33-col streams).
                    # 10 out groups: col 33j img q 0..127; col 33(4+j) img q
                    # 128..195 (68 valid + 60 spill); col 33(8+p) non-img of
                    # blocks 2p (rows 0:64) / 2p+1 (rows 64:128), 20 valid
                    # + 44 spill each.  Spill keeps all partitions defined
                    # so one recip + one mul normalizes everything.
                    pv = pvp.tile([128, 10 * VA], F32)
                    prev = None
                    for j, t in enumerate(ts):
                        has_c = _nvalid_core(t) > 128
                        co = c_off.get(j // 2)
                        b = 32 * (t % 2)
                        pr = t // 2
                        mm(pv[0:128, 33 * j:33 * j + VA],
                           probs[0:128, a_off[j]:a_off[j] + 128],
                           vp_sb[:, t, 0, :], start=True, stop=not has_c)
                        if has_c:
                            mm(pv[0:128, 33 * j:33 * j + VA],
                               probs[b:b + 25, co:co + 128],
                               vpc_sb[b:b + 25, pr, :],
                               start=False, stop=True)
                        mm(pv[0:128, 33 * (4 + j):33 * (4 + j) + VA],
                           probs[0:128, a_off[j] + 128:a_off[j] + 256],
                           vp_sb[:, t, 0, :], start=True, stop=not has_c)
                        if has_c:
                            mm(pv[0:128, 33 * (4 + j):33 * (4 + j) + VA],
                               probs[b:b + 25, co + 128:co + 256],
                               vpc_sb[b:b + 25, pr, :],
                               start=False, stop=True)
                        nch = _n_chunks(t)
                        qb = 64 * (j % 2)
                        for c in range(nch):
                            nco = n_cols[(j, c)]
                            mm(pv[qb:qb + 64, 33 * (8 + j // 2):33 * (8 + j // 2) + VA],
                               probs[0:128, nco:nco + 64],
                               vp_sb[0:128, t, c, :],
                               start=(c == 0), stop=(c == nch - 1))

                    # ---- normalize: one recip + one mul per quad
                    recips = recipsp.tile([128, 16], F32)
                    nc.vector.reciprocal(recips[0:128, 0:10],
                                         pv[0:128, 32:10 * VA:VA])
                    num_v = _strided2(pv[0:128, 0:1], (VA, 10), (1, 32))
                    rec_b = _strided2(recips[0:128, 0:1], (1, 10), (0, 32))
                    dst_v = _strided2(outst[0:128, 320 * g:320 * g + 1],
                                      (32, 10), (1, 32))
                    nc.vector.tensor_mul(dst_v, num_v, rec_b)

                    if g % 2 == 1:
                        hf = g // 2
                        nc.sync.dma_start(
                            out=out_d[i, :, 640 * hf:640 * hf + 640],
                            in_=outst[:, 640 * hf:640 * hf + 640])
    nc.compile()
    return nc


_NC = None


def _get_nc():
    global _NC
    if _NC is None:
        _NC = build_nc()
    return _NC


# ---------------------------------------------------------------- entry point

def kernel(q, k, v, feats_per_t, window_len, act_size, img_feat_size):
    assert int(feats_per_t) == F and int(window_len) == W
    assert int(act_size) == 16 and int(img_feat_size) == 196
    q = np.asarray(q, np.float32)
    k = np.asarray(k, np.float32)
    v = np.asarray(v, np.float32)

    packed = _pack_all(q, k, v)
    in_maps = []
    for core in range(N_CORES):
        s = slice(BH_PER_CORE * core, BH_PER_CORE * (core + 1))
        in_maps.append({n: np.ascontiguousarray(a[s]) for n, a in packed.items()})

    nc = _get_nc()
    res = run_bass_kernel_spmd(nc, in_maps, list(range(N_CORES)))
    out = np.empty((B * H, S, D), np.float32)
    for core in range(N_CORES):
        out[BH_PER_CORE * core:BH_PER_CORE * (core + 1)] = _unpack(
            res.results[core]["out"])
    return out.reshape(B, H, S, D)


# revision 26
# speedup vs baseline: 1.7352x; 1.0696x over previous
"""Trainium2 Bass kernel for nn_EyeRobotAgent block-sparse ("eye") attention.

Shapes: q,k,v [2, 12, 3456, 32] fp32.  S = 16 time-blocks x 216 feats.
Mask structure (per query block t):
  - img queries (m in [20,216), 196 of them) see only the "core" keys:
    19 keys (m in {0..3,5..19}) of each block tau in [t-7, t] plus m4(t)
    -> at most 153 keys,
  - non-img queries (m in [0,20), 20 of them) see core keys + the 196 img
    keys of block t (joint queries additionally lose past joint keys,
    handled by a bias row).

Strategy (data-parallel: 24 (b,h) pairs over 8 cores, 3 each):
  Per block pack keys as [core 153 | img 196 | pad] = 384 (newest-first so
  invalid tail cols are contiguous; masks fold into 2 bias contraction
  rows).  Scores are computed transposed [kv, q] in per-quad (4 block)
  PSUM tiles so a single ACT exp covers ~1000-1250 columns:
    N_j: 20 non-img queries vs 2-3 full-height 128-row chunks of the pack
    A_j: 196 img queries vs core[0:128]
    C:   core[128:153] tails of the 4 blocks packed into 32-row PE
         quadrant bands (tile_position rows 32j), one shared 196-col region
    PAD: 28 dummy cols kept defined so PV lhsT "spill" reads stay legal.
  PV uses probs as the stationary operand (out[q, 33] per matmul streams
  only 33 columns); the appended ones-column of V yields the softmax
  denominator in col 32; normalize is one DVE reciprocal + one mul per
  quad, padded out-groups making every partition defined.
  exp() has no max-subtraction (scores are O(6), fp32-safe).
"""
import math
import numpy as np

import concourse.bass as bass
import concourse.mybir as mybir
import concourse.tile as tile
from concourse import bacc
from concourse.bass_utils import run_bass_kernel_spmd
from concourse.tile_rust import add_dep_helper

B, H, S, D = 2, 12, 3456, 32
F = 216            # feats_per_t
W = 8              # window_len
T = S // F         # 16 blocks
IMG_START = 20     # F - img_feat_size
PAST_SEL = np.array([0, 1, 2, 3] + list(range(5, 20)))   # 19 per block
NCORE = 153        # 8*19 + 1 (m4) candidate core keys
NIMG = 196
PACK = 384         # [core (<=153, compact) | img 196 | pad]
KAUG = D + 3       # 35 = 32 d + joint-bias + img-img bias + validity rows
VA = D + 1         # 33 = v columns + ones column
NEG = np.float32(-30000.0)
SCALE = float(1.0 / np.sqrt(np.float32(D)))
N_CORES = 8
BH_PER_CORE = (B * H) // N_CORES      # 3
NQ = 4                                # blocks per quad
QUADS = T // NQ                       # 4

F32 = mybir.dt.float32
BF16 = mybir.dt.float16      # half precision: matmul rate 1 cyc/row
NP_BF16 = np.float16


def _nvalid_core(t):
    return 20 + 19 * min(t, 7)


def _n_chunks(t):
    return math.ceil((_nvalid_core(t) + NIMG) / 128)


def _pack_rows(t):
    """Compact key packing for block t: [19(t), m4(t), 19(t-1), ..,
    19(t-min(t,7)), img(t) 196, pad].  -1 marks invalid (trailing only)."""
    rows = list(F * t + PAST_SEL) + [F * t + 4]
    for s in range(1, min(t, 7) + 1):
        rows += list(F * (t - s) + PAST_SEL)
    rows += list(range(F * t + IMG_START, F * t + F))
    rows += [-1] * (PACK - len(rows))
    return np.array(rows)


def _quad_layout(g):
    """Column layout of the per-quad score tile, bank-aware (matmul outputs
    must not cross 512-col PSUM bank boundaries): bank0 = [A0 A1 N x6],
    bank1 = [A2 A3 N-rest pad?], bank2 = [C... pad].  C regions are per
    block PAIR (bands rows 0:32 / 32:64 by t%2, rows 64:128 dummy-filled).
    Exp'd cols [0:ncols] are gap-free; 60 pad cols keep PV spill reads
    defined."""
    ts = list(range(NQ * g, NQ * g + NQ))
    chunks = [(j, c) for j, t in enumerate(ts) for c in range(_n_chunks(t))]
    a_off = [0, 196, 512, 708]
    n_cols = {}
    col = 392
    for jc in chunks[:6]:
        n_cols[jc] = col
        col += 20
    assert col <= 512
    col = 904
    for jc in chunks[6:]:
        n_cols[jc] = col
        col += 20
    pads = []
    if col < 1024:
        pads.append((col, 1024 - col))
    c_off = {}
    cbase = 1024
    for pl in range(NQ // 2):           # local pair index
        if any(_nvalid_core(t) > 128 for t in ts[2 * pl:2 * pl + 2]):
            c_off[pl] = cbase
            cbase += NIMG
    pads.append((cbase, 60))
    return ts, n_cols, a_off, c_off, pads, cbase + 60


# ---------------------------------------------------------------- host packing

def _pack_all(q, k, v):
    nbh = B * H
    qf = q.reshape(nbh, S, D)
    kf = k.reshape(nbh, S, D)
    vf = v.reshape(nbh, S, D)
    qm = np.arange(S) % F

    qt = np.zeros((nbh, KAUG, S), np.float32)
    qt[:, :D] = qf.transpose(0, 2, 1)
    qt[:, 32] = (qm >= 4) & (qm < IMG_START)      # is_joint(q)
    qt[:, 33] = qm >= IMG_START                   # is_img(q)
    qt[:, 34] = 1.0

    kpt = np.zeros((nbh, KAUG, T, PACK), np.float32)
    vp = np.zeros((nbh, 128, T, 3, VA), np.float32)
    vpc = np.zeros((nbh, 128, T // 2, VA), np.float32)
    for t in range(T):
        rows = _pack_rows(t)
        valid = rows >= 0
        safe = np.where(valid, rows, 0)
        kpt[:, :D, t] = np.where(valid[None, None], kf[:, safe].transpose(0, 2, 1), 0.0)
        # joint-past bias: past sets s=1..min(t,7) at cols 20+19(s-1),
        # joint keys at positions 4..18 within each set
        jbias = np.zeros(PACK, np.float32)
        for s in range(1, min(t, 7) + 1):
            base = 20 + 19 * (s - 1)
            jbias[base + 4: base + 19] = NEG
        kpt[:, 32, t] = jbias
        kpt[:, 33, t] = np.where(valid & (rows % F >= IMG_START), NEG, 0.0)
        kpt[:, 34, t] = np.where(valid, 0.0, NEG)
        vblk = np.where(valid[None, :, None], vf[:, safe], 0.0)   # [nbh,384,32]
        vp[:, :, t, :, :D] = vblk.reshape(nbh, 3, 128, D).transpose(0, 2, 1, 3)
        # ones column: eps (not 0) on invalid rows keeps every PV spill-row
        # denominator strictly positive (invalid probs are exactly 0, so
        # valid outputs are unaffected).
        ones = np.where(valid, 1.0, 6e-5)
        vp[:, :, t, :, 32] = ones.reshape(3, 128).T[None]
        pr, b = t // 2, 32 * (t % 2)
        # C tail: only core positions 128..nvalid_core-1 (img keys that
        # fall in [128:153] of the compact pack must contribute zero)
        ncv = _nvalid_core(t)
        tail = safe[128:NCORE]
        tval = np.arange(128, NCORE) < ncv
        vpc[:, b:b + 25, pr, :D] = np.where(tval[None, :, None], vf[:, tail], 0.0)
        vpc[:, b:b + 25, pr, 32] = np.where(tval, 1.0, 0.0)
    return {"qt": qt.astype(NP_BF16),
            "kpt": np.ascontiguousarray(kpt.astype(NP_BF16)),
            "vp": np.ascontiguousarray(vp.astype(NP_BF16)),
            "vpc": np.ascontiguousarray(vpc.astype(NP_BF16))}


def _unpack(arr):
    """arr [n, 128, QUADS*320] staging -> [n, S, D].  Per quad g, 10 groups
    of 32 cols: j=0..3 img q 20..147 (rows 0:128); 4..7 img q 148..215
    (rows 0:68); 8..9 non-img q 0..19 of blocks 2p (rows 0:20) and 2p+1
    (rows 64:84)."""
    n = arr.shape[0]
    r = arr.reshape(n, 128, QUADS, 10, 32).transpose(0, 2, 3, 1, 4)
    out = np.empty((n, QUADS, NQ, F, D), np.float32)
    for j in range(NQ):
        out[:, :, j, IMG_START:148] = r[:, :, j, 0:128]
        out[:, :, j, 148:] = r[:, :, 4 + j, 0:68]
        out[:, :, j, :IMG_START] = r[:, :, 8 + j // 2, 64 * (j % 2):64 * (j % 2) + 20]
    return out.reshape(n, S, D)


# ---------------------------------------------------------------- bass kernel

def build_nc(n_bh=BH_PER_CORE):
    nc = bacc.Bacc(None, target_bir_lowering=False, debug=False)
    qt_d = nc.declare_dram_parameter("qt", [n_bh, KAUG, S], BF16, isOutput=False)
    kpt_d = nc.declare_dram_parameter("kpt", [n_bh, KAUG, T, PACK], BF16, isOutput=False)
    vp_d = nc.declare_dram_parameter("vp", [n_bh, 128, T, 3, VA], BF16, isOutput=False)
    vpc_d = nc.declare_dram_parameter("vpc", [n_bh, 128, T // 2, VA], BF16, isOutput=False)
    # out mirrors the SBUF staging tile exactly; host unpacks.
    out_d = nc.declare_dram_parameter("out", [n_bh, 128, QUADS * 320], F32, isOutput=True)

    def _strided2(ap, d1, d2):
        return bass.AP(tensor=ap.tensor, offset=ap.offset,
                       ap=[list(ap.ap[0]), list(d1), list(d2)])

    with tile.TileContext(nc) as tc:
        with (
            tc.tile_pool(name="singles", bufs=1) as singles,
            tc.tile_pool(name="qtp", bufs=3) as qtp,
            tc.tile_pool(name="kptp", bufs=3) as kptp,
            tc.tile_pool(name="vpp", bufs=3) as vpp,
            tc.tile_pool(name="vpcp", bufs=3) as vpcp,
            tc.tile_pool(name="probsp", bufs=3) as probsp,
            tc.tile_pool(name="recipsp", bufs=3) as recipsp,
            tc.tile_pool(name="outsbp", bufs=3) as outsbp,
            tc.tile_pool(name="scoresp", bufs=2, space="PSUM") as scoresp,
            tc.tile_pool(name="pvp", bufs=2, space="PSUM") as pvp,
        ):
            zero = singles.tile([1, 128], BF16)
            nc.vector.memset(zero[:], 0.0)

            for i in range(n_bh):
                qt_sb = qtp.tile([KAUG, S], BF16)
                kpt_sb = kptp.tile([KAUG, T, PACK], BF16)
                vp_sb = vpp.tile([128, T, 3, VA], BF16)
                vpc_sb = vpcp.tile([128, T // 2, VA], BF16)
                nc.sync.dma_start(out=qt_sb[:], in_=qt_d[i])
                nc.sync.dma_start(out=kpt_sb[:], in_=kpt_d[i])
                nc.sync.dma_start(out=vp_sb[:], in_=vp_d[i])
                nc.sync.dma_start(out=vpc_sb[:], in_=vpc_d[i])
                outst = outsbp.tile([128, QUADS * 320], F32)

                for g in range(QUADS):
                    ts, n_cols, a_off, c_off, pads, ncols = _quad_layout(g)

                    # ---- QK^T (transposed scores [kv, q]); masks via the 2
                    # bias rows; all mms chained (same-bank group ordering).
                    scores = scoresp.tile([128, 1536], F32)
                    prev = None

                    def mm(out, lhsT, rhs, start=True, stop=True):
                        nonlocal prev
                        m = nc.tensor.matmul(out, lhsT=lhsT, rhs=rhs,
                                             start=start, stop=stop)
                        if prev is not None:
                            add_dep_helper(m.ins, prev.ins, sync=False,
                                           reason="bank group order")
                        prev = m
                        return m

                    for j, t in enumerate(ts):
                        for c in range(_n_chunks(t)):
                            nco = n_cols[(j, c)]
                            mm(scores[0:128, nco:nco + 20],
                               kpt_sb[:, t, 128 * c:128 * c + 128],
                               qt_sb[:, F * t:F * t + 20])
                    for j, t in enumerate(ts):
                        mm(scores[0:128, a_off[j]:a_off[j] + NIMG],
                           kpt_sb[:, t, 0:128],
                           qt_sb[:, F * t + IMG_START:F * t + F])
                    for pl, co in c_off.items():
                        for t in ts[2 * pl:2 * pl + 2]:
                            b = 32 * (t % 2)
                            if _nvalid_core(t) > 128:
                                mm(scores[b:b + 32, co:co + NIMG],
                                   kpt_sb[:, t, 128:160],
                                   qt_sb[:, F * t + IMG_START:F * t + F])
                            else:      # dummy fill: zero scores -> probs 1
                                mm(scores[b:b + 32, co:co + NIMG],
                                   zero[0:1, 0:32], qt_sb[0:1, 0:NIMG])
                        mm(scores[64:128, co:co + NIMG],
                           zero[0:1, 0:64], qt_sb[0:1, 0:NIMG])
                    for poff, pw in pads:
                        mm(scores[0:128, poff:poff + pw],
                           zero[0:1, 0:128], qt_sb[0:1, 0:pw])

                    # ---- probs = exp(scale * scores), one ACT op per quad
                    probs = probsp.tile([128, 1536], BF16)
                    nc.scalar.activation(probs[0:128, 0:ncols],
                                         scores[0:128, 0:ncols],
                                         mybir.ActivationFunctionType.Exp,
                                         scale=SCALE)

                    # ---- PV: probs stationary, out[q, va] (33-col streams).
                    # 10 out groups: col 33j img q 0..127; col 33(4+j) img q
                    # 128..195 (68 valid + 60 spill); col 33(8+p) non-img of
                    # blocks 2p (rows 0:64) / 2p+1 (rows 64:128), 20 valid
                    # + 44 spill each.  Spill keeps all partitions defined
                    # so one recip + one mul normalizes everything.
                    pv = pvp.tile([128, 10 * VA], F32)
                    prev = None
                    for j, t in enumerate(ts):
                        has_c = _nvalid_core(t) > 128
                        co = c_off.get(j // 2)
                        b = 32 * (t % 2)
                        pr = t // 2
                        mm(pv[0:128, 33 * j:33 * j + VA],
                           probs[0:128, a_off[j]:a_off[j] + 128],
                           vp_sb[:, t, 0, :], start=True, stop=not has_c)
                        if has_c:
                            mm(pv[0:128, 33 * j:33 * j + VA],
                               probs[b:b + 25, co:co + 128],
                               vpc_sb[b:b + 25, pr, :],
                               start=False, stop=True)
                        mm(pv[0:128, 33 * (4 + j):33 * (4 + j) + VA],
                           probs[0:128, a_off[j] + 128:a_off[j] + 256],
                           vp_sb[:, t, 0, :], start=True, stop=not has_c)
                        if has_c:
                            mm(pv[0:128, 33 * (4 + j):33 * (4 + j) + VA],
                               probs[b:b + 25, co + 128:co + 256],
                               vpc_sb[b:b + 25, pr, :],
                               start=False, stop=True)
                        nch = _n_chunks(t)
                        qb = 64 * (j % 2)
                        for c in range(nch):
                            nco = n_cols[(j, c)]
                            mm(pv[qb:qb + 64, 33 * (8 + j // 2):33 * (8 + j // 2) + VA],
                               probs[0:128, nco:nco + 64],
                               vp_sb[0:128, t, c, :],
                               start=(c == 0), stop=(c == nch - 1))

                    # ---- normalize: one recip + one mul per quad
                    recips = recipsp.tile([128, 16], F32)
                    nc.vector.reciprocal(recips[0:128, 0:10],
                                         pv[0:128, 32:10 * VA:VA])
                    num_v = _strided2(pv[0:128, 0:1], (VA, 10), (1, 32))
                    rec_b = _strided2(recips[0:128, 0:1], (1, 10), (0, 32))
                    dst_v = _strided2(outst[0:128, 320 * g:320 * g + 1],
                                      (32, 10), (1, 32))
                    nc.vector.tensor_mul(dst_v, num_v, rec_b)

                    # out-DMA via the idle Pool engine's SWDGE queue so its
                    # data wait can't head-of-line-block input DMAs on SP
                    if g % 2 == 1:
                        hf = g // 2
                        nc.gpsimd.dma_start(
                            out=out_d[i, :, 640 * hf:640 * hf + 640],
                            in_=outst[:, 640 * hf:640 * hf + 640])
    nc.compile()
    return nc


_NC = None


def _get_nc():
    global _NC
    if _NC is None:
        _NC = build_nc()
    return _NC


# ---------------------------------------------------------------- entry point

def kernel(q, k, v, feats_per_t, window_len, act_size, img_feat_size):
    assert int(feats_per_t) == F and int(window_len) == W
    assert int(act_size) == 16 and int(img_feat_size) == 196
    q = np.asarray(q, np.float32)
    k = np.asarray(k, np.float32)
    v = np.asarray(v, np.float32)

    packed = _pack_all(q, k, v)
    in_maps = []
    for core in range(N_CORES):
        s = slice(BH_PER_CORE * core, BH_PER_CORE * (core + 1))
        in_maps.append({n: np.ascontiguousarray(a[s]) for n, a in packed.items()})

    nc = _get_nc()
    res = run_bass_kernel_spmd(nc, in_maps, list(range(N_CORES)))
    out = np.empty((B * H, S, D), np.float32)
    for core in range(N_CORES):
        out[BH_PER_CORE * core:BH_PER_CORE * (core + 1)] = _unpack(
            res.results[core]["out"])
    return out.reshape(B, H, S, D)


# revision 28
# speedup vs baseline: 2.4176x; 1.3932x over previous
"""Trainium2 Bass kernel for nn_EyeRobotAgent block-sparse ("eye") attention.

Shapes: q,k,v [2, 12, 3456, 32] fp32.  S = 16 time-blocks x 216 feats.
Mask structure (per query block t):
  - img queries (m in [20,216), 196 of them) see only the "core" keys:
    19 keys (m in {0..3,5..19}) of each block tau in [t-7, t] plus m4(t)
    -> at most 153 keys,
  - non-img queries (m in [0,20), 20 of them) see core keys + the 196 img
    keys of block t (joint queries additionally lose past joint keys,
    handled by a bias row).

Strategy (data-parallel: 24 (b,h) pairs over 8 cores, 3 each):
  Per block pack keys as [core 153 | img 196 | pad] = 384 (newest-first so
  invalid tail cols are contiguous; masks fold into 2 bias contraction
  rows).  Scores are computed transposed [kv, q] in per-quad (4 block)
  PSUM tiles so a single ACT exp covers ~1000-1250 columns:
    N_j: 20 non-img queries vs 2-3 full-height 128-row chunks of the pack
    A_j: 196 img queries vs core[0:128]
    C:   core[128:153] tails of the 4 blocks packed into 32-row PE
         quadrant bands (tile_position rows 32j), one shared 196-col region
    PAD: 28 dummy cols kept defined so PV lhsT "spill" reads stay legal.
  PV uses probs as the stationary operand (out[q, 33] per matmul streams
  only 33 columns); the appended ones-column of V yields the softmax
  denominator in col 32; normalize is one DVE reciprocal + one mul per
  quad, padded out-groups making every partition defined.
  exp() has no max-subtraction (scores are O(6), fp32-safe).
"""
import math
import numpy as np

import concourse.bass as bass
import concourse.mybir as mybir
import concourse.tile as tile
from concourse import bacc
from concourse.bass_utils import run_bass_kernel_spmd
from concourse.tile_rust import add_dep_helper

B, H, S, D = 2, 12, 3456, 32
F = 216            # feats_per_t
W = 8              # window_len
T = S // F         # 16 blocks
IMG_START = 20     # F - img_feat_size
PAST_SEL = np.array([0, 1, 2, 3] + list(range(5, 20)))   # 19 per block
NCORE = 153        # 8*19 + 1 (m4) candidate core keys
NIMG = 196
PACK = 384         # [core (<=153, compact) | img 196 | pad]
KAUG = D + 3       # 35 = 32 d + joint-bias + img-img bias + validity rows
VA = D + 1         # 33 = v columns + ones column
NEG = np.float32(-30000.0)
SCALE = float(1.0 / np.sqrt(np.float32(D)))
N_CORES = 8
BH_PER_CORE = (B * H) // N_CORES      # 3
NQ = 4                                # blocks per quad
QUADS = T // NQ                       # 4

F32 = mybir.dt.float32
BF16 = mybir.dt.float16      # half precision: matmul rate 1 cyc/row
NP_BF16 = np.float16


def _nvalid_core(t):
    return 20 + 19 * min(t, 7)


def _n_chunks(t):
    return math.ceil((_nvalid_core(t) + NIMG) / 128)


def _pack_rows(t):
    """Compact key packing for block t: [19(t), m4(t), 19(t-1), ..,
    19(t-min(t,7)), img(t) 196, pad].  -1 marks invalid (trailing only)."""
    rows = list(F * t + PAST_SEL) + [F * t + 4]
    for s in range(1, min(t, 7) + 1):
        rows += list(F * (t - s) + PAST_SEL)
    rows += list(range(F * t + IMG_START, F * t + F))
    rows += [-1] * (PACK - len(rows))
    return np.array(rows)


def _quad_layout(g):
    """Column layout of the per-quad score tile, bank-aware (matmul outputs
    must not cross 512-col PSUM bank boundaries): bank0 = [A0 A1 N x6],
    bank1 = [A2 A3 N-rest pad?], bank2 = [C... pad].  C regions are per
    block PAIR (bands rows 0:32 / 32:64 by t%2, rows 64:128 dummy-filled).
    Exp'd cols [0:ncols] are gap-free; 60 pad cols keep PV spill reads
    defined."""
    ts = list(range(NQ * g, NQ * g + NQ))
    chunks = [(j, c) for j, t in enumerate(ts) for c in range(_n_chunks(t))]
    a_off = [0, 196, 512, 708]
    n_cols = {}
    col = 392
    for jc in chunks[:6]:
        n_cols[jc] = col
        col += 20
    assert col <= 512
    col = 904
    for jc in chunks[6:]:
        n_cols[jc] = col
        col += 20
    pads = []
    if col < 1024:
        pads.append((col, 1024 - col))
    c_off = {}
    cbase = 1024
    for pl in range(NQ // 2):           # local pair index
        if any(_nvalid_core(t) > 128 for t in ts[2 * pl:2 * pl + 2]):
            c_off[pl] = cbase
            cbase += NIMG
    pads.append((cbase, 60))
    return ts, n_cols, a_off, c_off, pads, cbase + 60


# ---------------------------------------------------------------- host packing

def _pack_all(q, k, v):
    nbh = B * H
    qf = q.reshape(nbh, S, D)
    kf = k.reshape(nbh, S, D)
    vf = v.reshape(nbh, S, D)
    qm = np.arange(S) % F

    # qt/kpt live in two partition bands (rows 0:35 = blocks 0..7, rows
    # 64:99 = blocks 8..15): v1-model DMA cost is per-partition bytes, so
    # spreading over 2x partitions halves the transfer time.
    qtf = np.zeros((nbh, KAUG, S), np.float32)
    qtf[:, :D] = qf.transpose(0, 2, 1)
    qtf[:, 32] = (qm >= 4) & (qm < IMG_START)      # is_joint(q)
    qtf[:, 33] = qm >= IMG_START                   # is_img(q)
    qtf[:, 34] = 1.0
    qt = np.zeros((nbh, 99, S // 2), np.float32)
    qt[:, 0:KAUG] = qtf[:, :, 0:S // 2]
    qt[:, 64:64 + KAUG] = qtf[:, :, S // 2:]

    kpt = np.zeros((nbh, 99, T // 2, PACK), np.float32)
    vp = np.zeros((nbh, 128, T, 3, VA), np.float32)
    vpc = np.zeros((nbh, 128, T // 2, VA), np.float32)
    for t in range(T):
        rows = _pack_rows(t)
        valid = rows >= 0
        safe = np.where(valid, rows, 0)
        kb, tl = 64 * (t // 8), t % 8
        kpt[:, kb:kb + D, tl] = np.where(
            valid[None, None], kf[:, safe].transpose(0, 2, 1), 0.0)
        # joint-past bias: past sets s=1..min(t,7) at cols 20+19(s-1),
        # joint keys at positions 4..18 within each set
        jbias = np.zeros(PACK, np.float32)
        for s in range(1, min(t, 7) + 1):
            base = 20 + 19 * (s - 1)
            jbias[base + 4: base + 19] = NEG
        kpt[:, kb + 32, tl] = jbias
        kpt[:, kb + 33, tl] = np.where(valid & (rows % F >= IMG_START), NEG, 0.0)
        kpt[:, kb + 34, tl] = np.where(valid, 0.0, NEG)
        vblk = np.where(valid[None, :, None], vf[:, safe], 0.0)   # [nbh,384,32]
        vp[:, :, t, :, :D] = vblk.reshape(nbh, 3, 128, D).transpose(0, 2, 1, 3)
        # ones column: eps (not 0) on invalid rows keeps every PV spill-row
        # denominator strictly positive (invalid probs are exactly 0, so
        # valid outputs are unaffected).
        ones = np.where(valid, 1.0, 6e-5)
        vp[:, :, t, :, 32] = ones.reshape(3, 128).T[None]
        pr, b = t // 2, 32 * (t % 2)
        # C tail: only core positions 128..nvalid_core-1 (img keys that
        # fall in [128:153] of the compact pack must contribute zero)
        ncv = _nvalid_core(t)
        tail = safe[128:NCORE]
        tval = np.arange(128, NCORE) < ncv
        vpc[:, b:b + 25, pr, :D] = np.where(tval[None, :, None], vf[:, tail], 0.0)
        vpc[:, b:b + 25, pr, 32] = np.where(tval, 1.0, 0.0)
    # fold vpc after vp in one tensor (one DMA)
    vcomb = np.concatenate(
        [vp.reshape(nbh, 128, T * 3 * VA), vpc.reshape(nbh, 128, T // 2 * VA)],
        axis=2)
    return {"qt": np.ascontiguousarray(qt.astype(NP_BF16)),
            "kpt": np.ascontiguousarray(kpt.astype(NP_BF16)),
            "vp": np.ascontiguousarray(vcomb.astype(NP_BF16))}


def _unpack(arr):
    """arr [n, 128, QUADS*320] staging -> [n, S, D].  Per quad g, 10 groups
    of 32 cols: j=0..3 img q 20..147 (rows 0:128); 4..7 img q 148..215
    (rows 0:68); 8..9 non-img q 0..19 of blocks 2p (rows 0:20) and 2p+1
    (rows 64:84)."""
    n = arr.shape[0]
    arr = arr.astype(np.float32)
    r = arr.reshape(n, 128, QUADS, 10, 32).transpose(0, 2, 3, 1, 4)
    out = np.empty((n, QUADS, NQ, F, D), np.float32)
    for j in range(NQ):
        out[:, :, j, IMG_START:148] = r[:, :, j, 0:128]
        out[:, :, j, 148:] = r[:, :, 4 + j, 0:68]
        out[:, :, j, :IMG_START] = r[:, :, 8 + j // 2, 64 * (j % 2):64 * (j % 2) + 20]
    return out.reshape(n, S, D)


# ---------------------------------------------------------------- bass kernel

def build_nc(n_bh=BH_PER_CORE):
    nc = bacc.Bacc(None, target_bir_lowering=False, debug=False)
    qt_d = nc.declare_dram_parameter("qt", [n_bh, 99, S // 2], BF16, isOutput=False)
    kpt_d = nc.declare_dram_parameter("kpt", [n_bh, 99, T // 2, PACK], BF16, isOutput=False)
    vp_d = nc.declare_dram_parameter("vp", [n_bh, 128, (T * 3 + T // 2) * VA], BF16, isOutput=False)
    # out mirrors the SBUF staging tile exactly (fp16); host unpacks.
    out_d = nc.declare_dram_parameter("out", [n_bh, 128, QUADS * 320], BF16, isOutput=True)

    def _strided2(ap, d1, d2):
        return bass.AP(tensor=ap.tensor, offset=ap.offset,
                       ap=[list(ap.ap[0]), list(d1), list(d2)])

    with tile.TileContext(nc) as tc:
        with (
            tc.tile_pool(name="singles", bufs=1) as singles,
            tc.tile_pool(name="qtp", bufs=3) as qtp,
            tc.tile_pool(name="kptp", bufs=3) as kptp,
            tc.tile_pool(name="vpp", bufs=3) as vpp,
            tc.tile_pool(name="probsp", bufs=3) as probsp,
            tc.tile_pool(name="recipsp", bufs=3) as recipsp,
            tc.tile_pool(name="outsbp", bufs=3) as outsbp,
            tc.tile_pool(name="scoresp", bufs=2, space="PSUM") as scoresp,
            tc.tile_pool(name="pvp", bufs=2, space="PSUM") as pvp,
        ):
            zero = singles.tile([1, 128], BF16)
            nc.vector.memset(zero[:], 0.0)

            for i in range(n_bh):
                qt_sb = qtp.tile([99, S // 2], BF16)
                kpt_sb = kptp.tile([99, T // 2, PACK], BF16)
                vp_sb = vpp.tile([128, (T * 3 + T // 2) * VA], BF16)
                nc.sync.dma_start(out=qt_sb[:], in_=qt_d[i])
                nc.sync.dma_start(out=kpt_sb[:], in_=kpt_d[i])
                nc.gpsimd.dma_start(out=vp_sb[:], in_=vp_d[i])
                outst = outsbp.tile([128, QUADS * 320], BF16)

                VPC = T * 3 * VA          # vpc offset inside vp_sb

                def kslice(t, lo, hi):
                    return kpt_sb[64 * (t // 8):64 * (t // 8) + KAUG,
                                  t % 8, lo:hi]

                def qslice(t, lo, hi):
                    base = F * t - (S // 2) * (t // 8)
                    return qt_sb[64 * (t // 8):64 * (t // 8) + KAUG,
                                 base + lo:base + hi]

                def vslice(t, c):
                    return vp_sb[:, (3 * t + c) * VA:(3 * t + c + 1) * VA]

                for g in range(QUADS):
                    ts, n_cols, a_off, c_off, pads, ncols = _quad_layout(g)

                    # ---- QK^T (transposed scores [kv, q]); masks via the 2
                    # bias rows; all mms chained (same-bank group ordering).
                    scores = scoresp.tile([128, 1536], F32)
                    prev = None

                    def mm(out, lhsT, rhs, start=True, stop=True):
                        nonlocal prev
                        m = nc.tensor.matmul(out, lhsT=lhsT, rhs=rhs,
                                             start=start, stop=stop)
                        if prev is not None:
                            add_dep_helper(m.ins, prev.ins, sync=False,
                                           reason="bank group order")
                        prev = m
                        return m

                    for j, t in enumerate(ts):
                        for c in range(_n_chunks(t)):
                            nco = n_cols[(j, c)]
                            mm(scores[0:128, nco:nco + 20],
                               kslice(t, 128 * c, 128 * c + 128),
                               qslice(t, 0, IMG_START))
                    for j, t in enumerate(ts):
                        mm(scores[0:128, a_off[j]:a_off[j] + NIMG],
                           kslice(t, 0, 128),
                           qslice(t, IMG_START, F))
                    for pl, co in c_off.items():
                        for t in ts[2 * pl:2 * pl + 2]:
                            b = 32 * (t % 2)
                            if _nvalid_core(t) > 128:
                                mm(scores[b:b + 32, co:co + NIMG],
                                   kslice(t, 128, 160),
                                   qslice(t, IMG_START, F))
                            else:      # dummy fill: zero scores -> probs 1
                                mm(scores[b:b + 32, co:co + NIMG],
                                   zero[0:1, 0:32], qt_sb[0:1, 0:NIMG])
                        mm(scores[64:128, co:co + NIMG],
                           zero[0:1, 0:64], qt_sb[0:1, 0:NIMG])
                    for poff, pw in pads:
                        mm(scores[0:128, poff:poff + pw],
                           zero[0:1, 0:128], qt_sb[0:1, 0:pw])

                    # ---- probs = exp(scale * scores), one ACT op per quad
                    probs = probsp.tile([128, 1536], BF16)
                    nc.scalar.activation(probs[0:128, 0:ncols],
                                         scores[0:128, 0:ncols],
                                         mybir.ActivationFunctionType.Exp,
                                         scale=SCALE)

                    # ---- PV: probs stationary, out[q, va] (33-col streams).
                    # 10 out groups: col 33j img q 0..127; col 33(4+j) img q
                    # 128..195 (68 valid + 60 spill); col 33(8+p) non-img of
                    # blocks 2p (rows 0:64) / 2p+1 (rows 64:128), 20 valid
                    # + 44 spill each.  Spill keeps all partitions defined
                    # so one recip + one mul normalizes everything.
                    pv = pvp.tile([128, 10 * VA], F32)
                    prev = None
                    for j, t in enumerate(ts):
                        has_c = _nvalid_core(t) > 128
                        co = c_off.get(j // 2)
                        b = 32 * (t % 2)
                        pr = t // 2
                        mm(pv[0:128, 33 * j:33 * j + VA],
                           probs[0:128, a_off[j]:a_off[j] + 128],
                           vslice(t, 0), start=True, stop=not has_c)
                        if has_c:
                            mm(pv[0:128, 33 * j:33 * j + VA],
                               probs[b:b + 25, co:co + 128],
                               vp_sb[b:b + 25, VPC + pr * VA:VPC + (pr + 1) * VA],
                               start=False, stop=True)
                        mm(pv[0:128, 33 * (4 + j):33 * (4 + j) + VA],
                           probs[0:128, a_off[j] + 128:a_off[j] + 256],
                           vslice(t, 0), start=True, stop=not has_c)
                        if has_c:
                            mm(pv[0:128, 33 * (4 + j):33 * (4 + j) + VA],
                               probs[b:b + 25, co + 128:co + 256],
                               vp_sb[b:b + 25, VPC + pr * VA:VPC + (pr + 1) * VA],
                               start=False, stop=True)
                        nch = _n_chunks(t)
                        qb = 64 * (j % 2)
                        for c in range(nch):
                            nco = n_cols[(j, c)]
                            mm(pv[qb:qb + 64, 33 * (8 + j // 2):33 * (8 + j // 2) + VA],
                               probs[0:128, nco:nco + 64],
                               vslice(t, c),
                               start=(c == 0), stop=(c == nch - 1))

                    # ---- normalize: one recip + one mul per quad
                    recips = recipsp.tile([128, 16], F32)
                    nc.vector.reciprocal(recips[0:128, 0:10],
                                         pv[0:128, 32:10 * VA:VA])
                    num_v = _strided2(pv[0:128, 0:1], (VA, 10), (1, 32))
                    rec_b = _strided2(recips[0:128, 0:1], (1, 10), (0, 32))
                    dst_v = _strided2(outst[0:128, 320 * g:320 * g + 1],
                                      (32, 10), (1, 32))
                    nc.vector.tensor_mul(dst_v, num_v, rec_b)

                    # out-DMA via the idle Pool engine's SWDGE queue so its
                    # data wait can't head-of-line-block input DMAs on SP
                    if g % 2 == 1:
                        hf = g // 2
                        nc.gpsimd.dma_start(
                            out=out_d[i, :, 640 * hf:640 * hf + 640],
                            in_=outst[:, 640 * hf:640 * hf + 640])
    nc.compile()
    return nc


_NC = None


def _get_nc():
    global _NC
    if _NC is None:
        _NC = build_nc()
    return _NC


# ---------------------------------------------------------------- entry point

def kernel(q, k, v, feats_per_t, window_len, act_size, img_feat_size):
    assert int(feats_per_t) == F and int(window_len) == W
    assert int(act_size) == 16 and int(img_feat_size) == 196
    q = np.asarray(q, np.float32)
    k = np.asarray(k, np.float32)
    v = np.asarray(v, np.float32)

    packed = _pack_all(q, k, v)
    in_maps = []
    for core in range(N_CORES):
        s = slice(BH_PER_CORE * core, BH_PER_CORE * (core + 1))
        in_maps.append({n: np.ascontiguousarray(a[s]) for n, a in packed.items()})

    nc = _get_nc()
    res = run_bass_kernel_spmd(nc, in_maps, list(range(N_CORES)))
    out = np.empty((B * H, S, D), np.float32)
    for core in range(N_CORES):
        out[BH_PER_CORE * core:BH_PER_CORE * (core + 1)] = _unpack(
            res.results[core]["out"])
    return out.reshape(B, H, S, D)


# revision 29
# speedup vs baseline: 2.5931x; 1.0726x over previous
"""Trainium2 Bass kernel for nn_EyeRobotAgent block-sparse ("eye") attention.

Shapes: q,k,v [2, 12, 3456, 32] fp32.  S = 16 time-blocks x 216 feats.
Mask structure (per query block t):
  - img queries (m in [20,216), 196 of them) see only the "core" keys:
    19 keys (m in {0..3,5..19}) of each block tau in [t-7, t] plus m4(t)
    -> at most 153 keys,
  - non-img queries (m in [0,20), 20 of them) see core keys + the 196 img
    keys of block t (joint queries additionally lose past joint keys,
    handled by a bias row).

Strategy (data-parallel: 24 (b,h) pairs over 8 cores, 3 each):
  Per block pack keys as [core 153 | img 196 | pad] = 384 (newest-first so
  invalid tail cols are contiguous; masks fold into 2 bias contraction
  rows).  Scores are computed transposed [kv, q] in per-quad (4 block)
  PSUM tiles so a single ACT exp covers ~1000-1250 columns:
    N_j: 20 non-img queries vs 2-3 full-height 128-row chunks of the pack
    A_j: 196 img queries vs core[0:128]
    C:   core[128:153] tails of the 4 blocks packed into 32-row PE
         quadrant bands (tile_position rows 32j), one shared 196-col region
    PAD: 28 dummy cols kept defined so PV lhsT "spill" reads stay legal.
  PV uses probs as the stationary operand (out[q, 33] per matmul streams
  only 33 columns); the appended ones-column of V yields the softmax
  denominator in col 32; normalize is one DVE reciprocal + one mul per
  quad, padded out-groups making every partition defined.
  exp() has no max-subtraction (scores are O(6), fp32-safe).
"""
import math
import numpy as np

import concourse.bass as bass
import concourse.mybir as mybir
import concourse.tile as tile
from concourse import bacc
from concourse.bass_utils import run_bass_kernel_spmd
from concourse.tile_rust import add_dep_helper

B, H, S, D = 2, 12, 3456, 32
F = 216            # feats_per_t
W = 8              # window_len
T = S // F         # 16 blocks
IMG_START = 20     # F - img_feat_size
PAST_SEL = np.array([0, 1, 2, 3] + list(range(5, 20)))   # 19 per block
NCORE = 153        # 8*19 + 1 (m4) candidate core keys
NIMG = 196
PACK = 384         # [core (<=153, compact) | img 196 | pad]
KAUG = D + 3       # 35 = 32 d + joint-bias + img-img bias + validity rows
VA = D + 1         # 33 = v columns + ones column
NEG = np.float32(-30000.0)
SCALE = float(1.0 / np.sqrt(np.float32(D)))
N_CORES = 8
BH_PER_CORE = (B * H) // N_CORES      # 3
NQ = 4                                # blocks per quad
QUADS = T // NQ                       # 4

F32 = mybir.dt.float32
BF16 = mybir.dt.float16      # half precision: matmul rate 1 cyc/row
NP_BF16 = np.float16


def _nvalid_core(t):
    return 20 + 19 * min(t, 7)


def _n_chunks(t):
    return math.ceil((_nvalid_core(t) + NIMG) / 128)


def _pack_rows(t):
    """Compact key packing for block t: [19(t), m4(t), 19(t-1), ..,
    19(t-min(t,7)), img(t) 196, pad].  -1 marks invalid (trailing only)."""
    rows = list(F * t + PAST_SEL) + [F * t + 4]
    for s in range(1, min(t, 7) + 1):
        rows += list(F * (t - s) + PAST_SEL)
    rows += list(range(F * t + IMG_START, F * t + F))
    rows += [-1] * (PACK - len(rows))
    return np.array(rows)


def _quad_layout(g):
    """Column layout of the per-quad score tile, bank-aware (matmul outputs
    must not cross 512-col PSUM bank boundaries): bank0 = [A0 A1 N x6],
    bank1 = [A2 A3 N-rest pad?], bank2 = [C... pad].  C regions are per
    block PAIR (bands rows 0:32 / 32:64 by t%2, rows 64:128 dummy-filled).
    Exp'd cols [0:ncols] are gap-free; 60 pad cols keep PV spill reads
    defined."""
    ts = list(range(NQ * g, NQ * g + NQ))
    chunks = [(j, c) for j, t in enumerate(ts) for c in range(_n_chunks(t))]
    a_off = [0, 196, 512, 708]
    n_cols = {}
    col = 392
    for jc in chunks[:6]:
        n_cols[jc] = col
        col += 20
    assert col <= 512
    col = 904
    for jc in chunks[6:]:
        n_cols[jc] = col
        col += 20
    pads = []
    if col < 1024:
        pads.append((col, 1024 - col))
    c_off = {}
    cbase = 1024
    for pl in range(NQ // 2):           # local pair index
        if any(_nvalid_core(t) > 128 for t in ts[2 * pl:2 * pl + 2]):
            c_off[pl] = cbase
            cbase += NIMG
    pads.append((cbase, 60))
    return ts, n_cols, a_off, c_off, pads, cbase + 60


# ---------------------------------------------------------------- host packing

def _pack_all(q, k, v):
    nbh = B * H
    qf = q.reshape(nbh, S, D)
    kf = k.reshape(nbh, S, D)
    vf = v.reshape(nbh, S, D)
    qm = np.arange(S) % F

    # qt/kpt live in two partition bands (rows 0:35 = blocks 0..7, rows
    # 64:99 = blocks 8..15): v1-model DMA cost is per-partition bytes, so
    # spreading over 2x partitions halves the transfer time.
    qtf = np.zeros((nbh, KAUG, S), np.float32)
    qtf[:, :D] = qf.transpose(0, 2, 1)
    qtf[:, 32] = (qm >= 4) & (qm < IMG_START)      # is_joint(q)
    qtf[:, 33] = qm >= IMG_START                   # is_img(q)
    qtf[:, 34] = 1.0
    qt = np.zeros((nbh, 99, S // 2), np.float32)
    qt[:, 0:KAUG] = qtf[:, :, 0:S // 2]
    qt[:, 64:64 + KAUG] = qtf[:, :, S // 2:]

    kpt = np.zeros((nbh, 99, T // 2, PACK), np.float32)
    vp = np.zeros((nbh, 128, T, 3, VA), np.float32)
    vpc = np.zeros((nbh, 128, T // 2, VA), np.float32)
    for t in range(T):
        rows = _pack_rows(t)
        valid = rows >= 0
        safe = np.where(valid, rows, 0)
        kb, tl = 64 * (t // 8), t % 8
        kpt[:, kb:kb + D, tl] = np.where(
            valid[None, None], kf[:, safe].transpose(0, 2, 1), 0.0)
        # joint-past bias: past sets s=1..min(t,7) at cols 20+19(s-1),
        # joint keys at positions 4..18 within each set
        jbias = np.zeros(PACK, np.float32)
        for s in range(1, min(t, 7) + 1):
            base = 20 + 19 * (s - 1)
            jbias[base + 4: base + 19] = NEG
        kpt[:, kb + 32, tl] = jbias
        kpt[:, kb + 33, tl] = np.where(valid & (rows % F >= IMG_START), NEG, 0.0)
        kpt[:, kb + 34, tl] = np.where(valid, 0.0, NEG)
        vblk = np.where(valid[None, :, None], vf[:, safe], 0.0)   # [nbh,384,32]
        vp[:, :, t, :, :D] = vblk.reshape(nbh, 3, 128, D).transpose(0, 2, 1, 3)
        # ones column: eps (not 0) on invalid rows keeps every PV spill-row
        # denominator strictly positive (invalid probs are exactly 0, so
        # valid outputs are unaffected).
        ones = np.where(valid, 1.0, 6e-5)
        vp[:, :, t, :, 32] = ones.reshape(3, 128).T[None]
        pr, b = t // 2, 32 * (t % 2)
        # C tail: only core positions 128..nvalid_core-1 (img keys that
        # fall in [128:153] of the compact pack must contribute zero)
        ncv = _nvalid_core(t)
        tail = safe[128:NCORE]
        tval = np.arange(128, NCORE) < ncv
        vpc[:, b:b + 25, pr, :D] = np.where(tval[None, :, None], vf[:, tail], 0.0)
        vpc[:, b:b + 25, pr, 32] = np.where(tval, 1.0, 0.0)
    # fold vpc after vp in one tensor (one DMA)
    vcomb = np.concatenate(
        [vp.reshape(nbh, 128, T * 3 * VA), vpc.reshape(nbh, 128, T // 2 * VA)],
        axis=2)
    return {"qt": np.ascontiguousarray(qt.astype(NP_BF16)),
            "kpt": np.ascontiguousarray(kpt.astype(NP_BF16)),
            "vp": np.ascontiguousarray(vcomb.astype(NP_BF16))}


def _unpack(arr):
    """arr [n, 128, QUADS*320] staging -> [n, S, D].  Per quad g, 10 groups
    of 32 cols: j=0..3 img q 20..147 (rows 0:128); 4..7 img q 148..215
    (rows 0:68); 8..9 non-img q 0..19 of blocks 2p (rows 0:20) and 2p+1
    (rows 64:84)."""
    n = arr.shape[0]
    arr = arr.astype(np.float32)
    r = arr.reshape(n, 128, QUADS, 10, 32).transpose(0, 2, 3, 1, 4)
    out = np.empty((n, QUADS, NQ, F, D), np.float32)
    for j in range(NQ):
        out[:, :, j, IMG_START:148] = r[:, :, j, 0:128]
        out[:, :, j, 148:] = r[:, :, 4 + j, 0:68]
        out[:, :, j, :IMG_START] = r[:, :, 8 + j // 2, 64 * (j % 2):64 * (j % 2) + 20]
    return out.reshape(n, S, D)


# ---------------------------------------------------------------- bass kernel

def build_nc(n_bh=BH_PER_CORE):
    nc = bacc.Bacc(None, target_bir_lowering=False, debug=False)
    qt_d = nc.declare_dram_parameter("qt", [n_bh, 99, S // 2], BF16, isOutput=False)
    kpt_d = nc.declare_dram_parameter("kpt", [n_bh, 99, T // 2, PACK], BF16, isOutput=False)
    vp_d = nc.declare_dram_parameter("vp", [n_bh, 128, (T * 3 + T // 2) * VA], BF16, isOutput=False)
    # out mirrors the SBUF staging tile exactly (fp16); host unpacks.
    out_d = nc.declare_dram_parameter("out", [n_bh, 128, QUADS * 320], BF16, isOutput=True)

    def _strided2(ap, d1, d2):
        return bass.AP(tensor=ap.tensor, offset=ap.offset,
                       ap=[list(ap.ap[0]), list(d1), list(d2)])

    with tile.TileContext(nc) as tc:
        with (
            tc.tile_pool(name="singles", bufs=1) as singles,
            tc.tile_pool(name="qtp", bufs=3) as qtp,
            tc.tile_pool(name="kptp", bufs=3) as kptp,
            tc.tile_pool(name="vpp", bufs=3) as vpp,
            tc.tile_pool(name="probsp", bufs=3) as probsp,
            tc.tile_pool(name="recipsp", bufs=3) as recipsp,
            tc.tile_pool(name="outsbp", bufs=3) as outsbp,
            tc.tile_pool(name="scoresp", bufs=2, space="PSUM") as scoresp,
            tc.tile_pool(name="pvp", bufs=2, space="PSUM") as pvp,
        ):
            zero = singles.tile([1, 128], BF16)
            nc.vector.memset(zero[:], 0.0)

            for i in range(n_bh):
                qt_sb = qtp.tile([99, S // 2], BF16)
                kpt_sb = kptp.tile([99, T // 2, PACK], BF16)
                vp_sb = vpp.tile([128, (T * 3 + T // 2) * VA], BF16)
                # halves so quad 0 can start after ~half the input latency
                VH = T * 3 * VA // 2
                for hf in range(2):
                    qh, th = S // 4 * hf, T // 4 * hf
                    nc.sync.dma_start(out=qt_sb[:, qh:qh + S // 4],
                                      in_=qt_d[i, :, qh:qh + S // 4])
                    nc.sync.dma_start(out=kpt_sb[:, th:th + T // 4, :],
                                      in_=kpt_d[i, :, th:th + T // 4, :])
                    if hf == 0:
                        nc.gpsimd.dma_start(out=vp_sb[:, 0:VH],
                                            in_=vp_d[i, :, 0:VH])
                    else:
                        nc.gpsimd.dma_start(
                            out=vp_sb[:, VH:],
                            in_=vp_d[i, :, VH:])
                outst = outsbp.tile([128, QUADS * 320], BF16)

                VPC = T * 3 * VA          # vpc offset inside vp_sb

                def kslice(t, lo, hi):
                    return kpt_sb[64 * (t // 8):64 * (t // 8) + KAUG,
                                  t % 8, lo:hi]

                def qslice(t, lo, hi):
                    base = F * t - (S // 2) * (t // 8)
                    return qt_sb[64 * (t // 8):64 * (t // 8) + KAUG,
                                 base + lo:base + hi]

                def vslice(t, c):
                    return vp_sb[:, (3 * t + c) * VA:(3 * t + c + 1) * VA]

                for g in range(QUADS):
                    ts, n_cols, a_off, c_off, pads, ncols = _quad_layout(g)

                    # ---- QK^T (transposed scores [kv, q]); masks via the 2
                    # bias rows; all mms chained (same-bank group ordering).
                    scores = scoresp.tile([128, 1536], F32)
                    prev = None

                    def mm(out, lhsT, rhs, start=True, stop=True):
                        nonlocal prev
                        m = nc.tensor.matmul(out, lhsT=lhsT, rhs=rhs,
                                             start=start, stop=stop)
                        if prev is not None:
                            add_dep_helper(m.ins, prev.ins, sync=False,
                                           reason="bank group order")
                        prev = m
                        return m

                    for j, t in enumerate(ts):
                        for c in range(_n_chunks(t)):
                            nco = n_cols[(j, c)]
                            mm(scores[0:128, nco:nco + 20],
                               kslice(t, 128 * c, 128 * c + 128),
                               qslice(t, 0, IMG_START))
                    for j, t in enumerate(ts):
                        mm(scores[0:128, a_off[j]:a_off[j] + NIMG],
                           kslice(t, 0, 128),
                           qslice(t, IMG_START, F))
                    for pl, co in c_off.items():
                        for t in ts[2 * pl:2 * pl + 2]:
                            b = 32 * (t % 2)
                            if _nvalid_core(t) > 128:
                                mm(scores[b:b + 32, co:co + NIMG],
                                   kslice(t, 128, 160),
                                   qslice(t, IMG_START, F))
                            else:      # dummy fill: zero scores -> probs 1
                                mm(scores[b:b + 32, co:co + NIMG],
                                   zero[0:1, 0:32], qt_sb[0:1, 0:NIMG])
                        mm(scores[64:128, co:co + NIMG],
                           zero[0:1, 0:64], qt_sb[0:1, 0:NIMG])
                    for poff, pw in pads:
                        mm(scores[0:128, poff:poff + pw],
                           zero[0:1, 0:128], qt_sb[0:1, 0:pw])

                    # ---- probs = exp(scale * scores), one ACT op per quad
                    probs = probsp.tile([128, 1536], BF16)
                    nc.scalar.activation(probs[0:128, 0:ncols],
                                         scores[0:128, 0:ncols],
                                         mybir.ActivationFunctionType.Exp,
                                         scale=SCALE)

                    # ---- PV: probs stationary, out[q, va] (33-col streams).
                    # 10 out groups: col 33j img q 0..127; col 33(4+j) img q
                    # 128..195 (68 valid + 60 spill); col 33(8+p) non-img of
                    # blocks 2p (rows 0:64) / 2p+1 (rows 64:128), 20 valid
                    # + 44 spill each.  Spill keeps all partitions defined
                    # so one recip + one mul normalizes everything.
                    pv = pvp.tile([128, 10 * VA], F32)
                    prev = None
                    for j, t in enumerate(ts):
                        has_c = _nvalid_core(t) > 128
                        co = c_off.get(j // 2)
                        b = 32 * (t % 2)
                        pr = t // 2
                        mm(pv[0:128, 33 * j:33 * j + VA],
                           probs[0:128, a_off[j]:a_off[j] + 128],
                           vslice(t, 0), start=True, stop=not has_c)
                        if has_c:
                            mm(pv[0:128, 33 * j:33 * j + VA],
                               probs[b:b + 25, co:co + 128],
                               vp_sb[b:b + 25, VPC + pr * VA:VPC + (pr + 1) * VA],
                               start=False, stop=True)
                        mm(pv[0:128, 33 * (4 + j):33 * (4 + j) + VA],
                           probs[0:128, a_off[j] + 128:a_off[j] + 256],
                           vslice(t, 0), start=True, stop=not has_c)
                        if has_c:
                            mm(pv[0:128, 33 * (4 + j):33 * (4 + j) + VA],
                               probs[b:b + 25, co + 128:co + 256],
                               vp_sb[b:b + 25, VPC + pr * VA:VPC + (pr + 1) * VA],
                               start=False, stop=True)
                        nch = _n_chunks(t)
                        qb = 64 * (j % 2)
                        for c in range(nch):
                            nco = n_cols[(j, c)]
                            mm(pv[qb:qb + 64, 33 * (8 + j // 2):33 * (8 + j // 2) + VA],
                               probs[0:128, nco:nco + 64],
                               vslice(t, c),
                               start=(c == 0), stop=(c == nch - 1))

                    # ---- normalize: one recip + one mul per quad
                    recips = recipsp.tile([128, 16], F32)
                    nc.vector.reciprocal(recips[0:128, 0:10],
                                         pv[0:128, 32:10 * VA:VA])
                    num_v = _strided2(pv[0:128, 0:1], (VA, 10), (1, 32))
                    rec_b = _strided2(recips[0:128, 0:1], (1, 10), (0, 32))
                    dst_v = _strided2(outst[0:128, 320 * g:320 * g + 1],
                                      (32, 10), (1, 32))
                    nc.vector.tensor_mul(dst_v, num_v, rec_b)

                    # out-DMA per quad via the idle Pool engine's SWDGE
                    # queue so its data wait can't block input DMAs on SP
                    nc.gpsimd.dma_start(
                        out=out_d[i, :, 320 * g:320 * g + 320],
                        in_=outst[:, 320 * g:320 * g + 320])
    nc.compile()
    return nc


_NC = None


def _get_nc():
    global _NC
    if _NC is None:
        _NC = build_nc()
    return _NC


# ---------------------------------------------------------------- entry point

def kernel(q, k, v, feats_per_t, window_len, act_size, img_feat_size):
    assert int(feats_per_t) == F and int(window_len) == W
    assert int(act_size) == 16 and int(img_feat_size) == 196
    q = np.asarray(q, np.float32)
    k = np.asarray(k, np.float32)
    v = np.asarray(v, np.float32)

    packed = _pack_all(q, k, v)
    in_maps = []
    for core in range(N_CORES):
        s = slice(BH_PER_CORE * core, BH_PER_CORE * (core + 1))
        in_maps.append({n: np.ascontiguousarray(a[s]) for n, a in packed.items()})

    nc = _get_nc()
    res = run_bass_kernel_spmd(nc, in_maps, list(range(N_CORES)))
    out = np.empty((B * H, S, D), np.float32)
    for core in range(N_CORES):
        out[BH_PER_CORE * core:BH_PER_CORE * (core + 1)] = _unpack(
            res.results[core]["out"])
    return out.reshape(B, H, S, D)


# revision 30
# speedup vs baseline: 2.8656x; 1.1051x over previous
"""Trainium2 Bass kernel for nn_EyeRobotAgent block-sparse ("eye") attention.

Shapes: q,k,v [2, 12, 3456, 32] fp32.  S = 16 time-blocks x 216 feats.
Mask structure (per query block t):
  - img queries (m in [20,216), 196 of them) see only the "core" keys:
    19 keys (m in {0..3,5..19}) of each block tau in [t-7, t] plus m4(t)
    -> at most 153 keys,
  - non-img queries (m in [0,20), 20 of them) see core keys + the 196 img
    keys of block t (joint queries additionally lose past joint keys,
    handled by a bias row).

Strategy (data-parallel: 24 (b,h) pairs over 8 cores, 3 each):
  Per block pack keys as [core 153 | img 196 | pad] = 384 (newest-first so
  invalid tail cols are contiguous; masks fold into 2 bias contraction
  rows).  Scores are computed transposed [kv, q] in per-quad (4 block)
  PSUM tiles so a single ACT exp covers ~1000-1250 columns:
    N_j: 20 non-img queries vs 2-3 full-height 128-row chunks of the pack
    A_j: 196 img queries vs core[0:128]
    C:   core[128:153] tails of the 4 blocks packed into 32-row PE
         quadrant bands (tile_position rows 32j), one shared 196-col region
    PAD: 28 dummy cols kept defined so PV lhsT "spill" reads stay legal.
  PV uses probs as the stationary operand (out[q, 33] per matmul streams
  only 33 columns); the appended ones-column of V yields the softmax
  denominator in col 32; normalize is one DVE reciprocal + one mul per
  quad, padded out-groups making every partition defined.
  exp() has no max-subtraction (scores are O(6), fp32-safe).
"""
import math
import numpy as np

import concourse.bass as bass
import concourse.mybir as mybir
import concourse.tile as tile
from concourse import bacc
from concourse.bass_utils import run_bass_kernel_spmd
from concourse.tile_rust import add_dep_helper

B, H, S, D = 2, 12, 3456, 32
F = 216            # feats_per_t
W = 8              # window_len
T = S // F         # 16 blocks
IMG_START = 20     # F - img_feat_size
PAST_SEL = np.array([0, 1, 2, 3] + list(range(5, 20)))   # 19 per block
NCORE = 153        # 8*19 + 1 (m4) candidate core keys
NIMG = 196
PACK = 384         # [core (<=153, compact) | img 196 | pad]
KAUG = D + 3       # 35 = 32 d + joint-bias + img-img bias + validity rows
VA = D + 1         # 33 = v columns + ones column
NEG = np.float32(-30000.0)
SCALE = float(1.0 / np.sqrt(np.float32(D)))
N_CORES = 8
BH_PER_CORE = (B * H) // N_CORES      # 3
NQ = 4                                # blocks per quad
QUADS = T // NQ                       # 4

F32 = mybir.dt.float32
BF16 = mybir.dt.float16      # half precision: matmul rate 1 cyc/row
NP_BF16 = np.float16


def _nvalid_core(t):
    return 20 + 19 * min(t, 7)


def _n_chunks(t):
    return math.ceil((_nvalid_core(t) + NIMG) / 128)


def _pack_rows(t):
    """Compact key packing for block t: [19(t), m4(t), 19(t-1), ..,
    19(t-min(t,7)), img(t) 196, pad].  -1 marks invalid (trailing only)."""
    rows = list(F * t + PAST_SEL) + [F * t + 4]
    for s in range(1, min(t, 7) + 1):
        rows += list(F * (t - s) + PAST_SEL)
    rows += list(range(F * t + IMG_START, F * t + F))
    rows += [-1] * (PACK - len(rows))
    return np.array(rows)


def _quad_layout(g):
    """Column layout of the per-quad score tile, bank-aware (matmul outputs
    must not cross 512-col PSUM bank boundaries): bank0 = [A0 A1 N x6],
    bank1 = [A2 A3 N-rest pad?], bank2 = [C... pad].  C regions are per
    block PAIR (bands rows 0:32 / 32:64 by t%2, rows 64:128 dummy-filled).
    Exp'd cols [0:ncols] are gap-free; 60 pad cols keep PV spill reads
    defined."""
    ts = list(range(NQ * g, NQ * g + NQ))
    chunks = [(j, c) for j, t in enumerate(ts) for c in range(_n_chunks(t))]
    a_off = [0, 196, 512, 708]
    n_cols = {}
    col = 392
    for jc in chunks[:6]:
        n_cols[jc] = col
        col += 20
    assert col <= 512
    col = 904
    for jc in chunks[6:]:
        n_cols[jc] = col
        col += 20
    pads = []
    if col < 1024:
        pads.append((col, 1024 - col))
    c_off = {}
    cbase = 1024
    for pl in range(NQ // 2):           # local pair index
        if any(_nvalid_core(t) > 128 for t in ts[2 * pl:2 * pl + 2]):
            c_off[pl] = cbase
            cbase += NIMG
    pads.append((cbase, 60))
    return ts, n_cols, a_off, c_off, pads, cbase + 60


# ---------------------------------------------------------------- host packing

def _pack_all(q, k, v):
    nbh = B * H
    qf = q.reshape(nbh, S, D)
    kf = k.reshape(nbh, S, D)
    vf = v.reshape(nbh, S, D)
    qm = np.arange(S) % F

    # qt/kpt live in two partition bands (rows 0:35 = blocks 0..7, rows
    # 64:99 = blocks 8..15): v1-model DMA cost is per-partition bytes, so
    # spreading over 2x partitions halves the transfer time.
    qtf = np.zeros((nbh, KAUG, S), np.float32)
    qtf[:, :D] = qf.transpose(0, 2, 1)
    qtf[:, 32] = (qm >= 4) & (qm < IMG_START)      # is_joint(q)
    qtf[:, 33] = qm >= IMG_START                   # is_img(q)
    qtf[:, 34] = 1.0
    qt = np.zeros((nbh, 99, S // 2), np.float32)
    qt[:, 0:KAUG] = qtf[:, :, 0:S // 2]
    qt[:, 64:64 + KAUG] = qtf[:, :, S // 2:]

    kpt = np.zeros((nbh, 99, T // 2, PACK), np.float32)
    vp = np.zeros((nbh, 128, T, 3, VA), np.float32)
    vpc = np.zeros((nbh, 128, T // 2, VA), np.float32)
    for t in range(T):
        rows = _pack_rows(t)
        valid = rows >= 0
        safe = np.where(valid, rows, 0)
        kb, tl = 64 * (t // 8), t % 8
        kpt[:, kb:kb + D, tl] = np.where(
            valid[None, None], kf[:, safe].transpose(0, 2, 1), 0.0)
        # joint-past bias: past sets s=1..min(t,7) at cols 20+19(s-1),
        # joint keys at positions 4..18 within each set
        jbias = np.zeros(PACK, np.float32)
        for s in range(1, min(t, 7) + 1):
            base = 20 + 19 * (s - 1)
            jbias[base + 4: base + 19] = NEG
        kpt[:, kb + 32, tl] = jbias
        kpt[:, kb + 33, tl] = np.where(valid & (rows % F >= IMG_START), NEG, 0.0)
        kpt[:, kb + 34, tl] = np.where(valid, 0.0, NEG)
        vblk = np.where(valid[None, :, None], vf[:, safe], 0.0)   # [nbh,384,32]
        vp[:, :, t, :, :D] = vblk.reshape(nbh, 3, 128, D).transpose(0, 2, 1, 3)
        # ones column: eps (not 0) on invalid rows keeps every PV spill-row
        # denominator strictly positive (invalid probs are exactly 0, so
        # valid outputs are unaffected).
        ones = np.where(valid, 1.0, 6e-5)
        vp[:, :, t, :, 32] = ones.reshape(3, 128).T[None]
        pr, b = t // 2, 32 * (t % 2)
        # C tail: only core positions 128..nvalid_core-1 (img keys that
        # fall in [128:153] of the compact pack must contribute zero)
        ncv = _nvalid_core(t)
        tail = safe[128:NCORE]
        tval = np.arange(128, NCORE) < ncv
        vpc[:, b:b + 25, pr, :D] = np.where(tval[None, :, None], vf[:, tail], 0.0)
        vpc[:, b:b + 25, pr, 32] = np.where(tval, 1.0, 0.0)
    # fold vpc after vp in one tensor (one DMA)
    vcomb = np.concatenate(
        [vp.reshape(nbh, 128, T * 3 * VA), vpc.reshape(nbh, 128, T // 2 * VA)],
        axis=2)
    return {"qt": np.ascontiguousarray(qt.astype(NP_BF16)),
            "kpt": np.ascontiguousarray(kpt.astype(NP_BF16)),
            "vp": np.ascontiguousarray(vcomb.astype(NP_BF16))}


def _unpack(arr):
    """arr [n, 128, QUADS*320] staging -> [n, S, D].  Per quad g, 10 groups
    of 32 cols: j=0..3 img q 20..147 (rows 0:128); 4..7 img q 148..215
    (rows 0:68); 8..9 non-img q 0..19 of blocks 2p (rows 0:20) and 2p+1
    (rows 64:84)."""
    n = arr.shape[0]
    arr = arr.astype(np.float32)
    r = arr.reshape(n, 128, QUADS, 10, 32).transpose(0, 2, 3, 1, 4)
    out = np.empty((n, QUADS, NQ, F, D), np.float32)
    for j in range(NQ):
        out[:, :, j, IMG_START:148] = r[:, :, j, 0:128]
        out[:, :, j, 148:] = r[:, :, 4 + j, 0:68]
        out[:, :, j, :IMG_START] = r[:, :, 8 + j // 2, 64 * (j % 2):64 * (j % 2) + 20]
    return out.reshape(n, S, D)


# ---------------------------------------------------------------- bass kernel

def build_nc(n_bh=BH_PER_CORE):
    nc = bacc.Bacc(None, target_bir_lowering=False, debug=False)
    qt_d = nc.declare_dram_parameter("qt", [n_bh, 99, S // 2], BF16, isOutput=False)
    kpt_d = nc.declare_dram_parameter("kpt", [n_bh, 99, T // 2, PACK], BF16, isOutput=False)
    vp_d = nc.declare_dram_parameter("vp", [n_bh, 128, (T * 3 + T // 2) * VA], BF16, isOutput=False)
    # out mirrors the SBUF staging tile exactly (fp16); host unpacks.
    out_d = nc.declare_dram_parameter("out", [n_bh, 128, QUADS * 320], BF16, isOutput=True)

    def _strided2(ap, d1, d2):
        return bass.AP(tensor=ap.tensor, offset=ap.offset,
                       ap=[list(ap.ap[0]), list(d1), list(d2)])

    with tile.TileContext(nc) as tc:
        with (
            tc.tile_pool(name="singles", bufs=1) as singles,
            tc.tile_pool(name="qtp", bufs=3) as qtp,
            tc.tile_pool(name="kptp", bufs=3) as kptp,
            tc.tile_pool(name="vpp", bufs=3) as vpp,
            tc.tile_pool(name="probsp", bufs=3) as probsp,
            tc.tile_pool(name="recipsp", bufs=3) as recipsp,
            tc.tile_pool(name="outsbp", bufs=3) as outsbp,
            tc.tile_pool(name="scoresp", bufs=2, space="PSUM") as scoresp,
            tc.tile_pool(name="pvp", bufs=2, space="PSUM") as pvp,
        ):
            zero = singles.tile([1, 128], BF16)
            nc.vector.memset(zero[:], 0.0)

            pending_outs = []
            for i in range(n_bh):
                qt_sb = qtp.tile([99, S // 2], BF16)
                kpt_sb = kptp.tile([99, T // 2, PACK], BF16)
                vp_sb = vpp.tile([128, (T * 3 + T // 2) * VA], BF16)
                # halves so quad 0 can start after ~half the input latency;
                # kpt.h1 goes on the Pool queue, parallel with qt.h1 on SP.
                # Prior-bh out-DMAs are emitted AFTER this bh's inputs so
                # they never head-of-line-block the input stream.
                VH = T * 3 * VA // 2
                nc.gpsimd.dma_start(out=kpt_sb[:, 0:T // 4, :],
                                    in_=kpt_d[i, :, 0:T // 4, :])
                nc.sync.dma_start(out=qt_sb[:, 0:S // 4],
                                  in_=qt_d[i, :, 0:S // 4])
                nc.sync.dma_start(out=qt_sb[:, S // 4:],
                                  in_=qt_d[i, :, S // 4:])
                nc.sync.dma_start(out=kpt_sb[:, T // 4:, :],
                                  in_=kpt_d[i, :, T // 4:, :])
                nc.gpsimd.dma_start(out=vp_sb[:, 0:VH], in_=vp_d[i, :, 0:VH])
                nc.gpsimd.dma_start(out=vp_sb[:, VH:], in_=vp_d[i, :, VH:])
                for fn in pending_outs:
                    fn()
                pending_outs = []
                outst = outsbp.tile([128, QUADS * 320], BF16)

                VPC = T * 3 * VA          # vpc offset inside vp_sb

                def kslice(t, lo, hi):
                    return kpt_sb[64 * (t // 8):64 * (t // 8) + KAUG,
                                  t % 8, lo:hi]

                def qslice(t, lo, hi):
                    base = F * t - (S // 2) * (t // 8)
                    return qt_sb[64 * (t // 8):64 * (t // 8) + KAUG,
                                 base + lo:base + hi]

                def vslice(t, c):
                    return vp_sb[:, (3 * t + c) * VA:(3 * t + c + 1) * VA]

                for g in range(QUADS):
                    ts, n_cols, a_off, c_off, pads, ncols = _quad_layout(g)

                    # ---- QK^T (transposed scores [kv, q]); masks via the 2
                    # bias rows; all mms chained (same-bank group ordering).
                    scores = scoresp.tile([128, 1536], F32)
                    prev = None

                    def mm(out, lhsT, rhs, start=True, stop=True):
                        nonlocal prev
                        m = nc.tensor.matmul(out, lhsT=lhsT, rhs=rhs,
                                             start=start, stop=stop)
                        if prev is not None:
                            add_dep_helper(m.ins, prev.ins, sync=False,
                                           reason="bank group order")
                        prev = m
                        return m

                    for j, t in enumerate(ts):
                        for c in range(_n_chunks(t)):
                            nco = n_cols[(j, c)]
                            mm(scores[0:128, nco:nco + 20],
                               kslice(t, 128 * c, 128 * c + 128),
                               qslice(t, 0, IMG_START))
                    for j, t in enumerate(ts):
                        mm(scores[0:128, a_off[j]:a_off[j] + NIMG],
                           kslice(t, 0, 128),
                           qslice(t, IMG_START, F))
                    for pl, co in c_off.items():
                        for t in ts[2 * pl:2 * pl + 2]:
                            b = 32 * (t % 2)
                            if _nvalid_core(t) > 128:
                                mm(scores[b:b + 32, co:co + NIMG],
                                   kslice(t, 128, 160),
                                   qslice(t, IMG_START, F))
                            else:      # dummy fill: zero scores -> probs 1
                                mm(scores[b:b + 32, co:co + NIMG],
                                   zero[0:1, 0:32], qt_sb[0:1, 0:NIMG])
                        mm(scores[64:128, co:co + NIMG],
                           zero[0:1, 0:64], qt_sb[0:1, 0:NIMG])


                    # ---- probs = exp(scale * scores), one ACT op per
                    # quad; trailing pad cols (PV spill targets) don't need
                    # exp -- Pool memsets them to 1.0 directly.
                    probs = probsp.tile([128, 1536], BF16)
                    pad0 = pads[0][0]
                    nc.scalar.activation(probs[0:128, 0:pad0],
                                         scores[0:128, 0:pad0],
                                         mybir.ActivationFunctionType.Exp,
                                         scale=SCALE)
                    nc.gpsimd.memset(probs[0:128, pad0:ncols], 1.0)

                    # ---- PV: probs stationary, out[q, va] (33-col streams).
                    # 10 out groups: col 33j img q 0..127; col 33(4+j) img q
                    # 128..195 (68 valid + 60 spill); col 33(8+p) non-img of
                    # blocks 2p (rows 0:64) / 2p+1 (rows 64:128), 20 valid
                    # + 44 spill each.  Spill keeps all partitions defined
                    # so one recip + one mul normalizes everything.
                    pv = pvp.tile([128, 10 * VA], F32)
                    prev = None
                    for j, t in enumerate(ts):
                        has_c = _nvalid_core(t) > 128
                        co = c_off.get(j // 2)
                        b = 32 * (t % 2)
                        pr = t // 2
                        mm(pv[0:128, 33 * j:33 * j + VA],
                           probs[0:128, a_off[j]:a_off[j] + 128],
                           vslice(t, 0), start=True, stop=not has_c)
                        if has_c:
                            mm(pv[0:128, 33 * j:33 * j + VA],
                               probs[b:b + 25, co:co + 128],
                               vp_sb[b:b + 25, VPC + pr * VA:VPC + (pr + 1) * VA],
                               start=False, stop=True)
                        mm(pv[0:128, 33 * (4 + j):33 * (4 + j) + VA],
                           probs[0:128, a_off[j] + 128:a_off[j] + 256],
                           vslice(t, 0), start=True, stop=not has_c)
                        if has_c:
                            mm(pv[0:128, 33 * (4 + j):33 * (4 + j) + VA],
                               probs[b:b + 25, co + 128:co + 256],
                               vp_sb[b:b + 25, VPC + pr * VA:VPC + (pr + 1) * VA],
                               start=False, stop=True)
                        nch = _n_chunks(t)
                        qb = 64 * (j % 2)
                        for c in range(nch):
                            nco = n_cols[(j, c)]
                            mm(pv[qb:qb + 64, 33 * (8 + j // 2):33 * (8 + j // 2) + VA],
                               probs[0:128, nco:nco + 64],
                               vslice(t, c),
                               start=(c == 0), stop=(c == nch - 1))

                    # ---- normalize: one recip + one mul per quad
                    recips = recipsp.tile([128, 16], F32)
                    nc.vector.reciprocal(recips[0:128, 0:10],
                                         pv[0:128, 32:10 * VA:VA])
                    num_v = _strided2(pv[0:128, 0:1], (VA, 10), (1, 32))
                    rec_b = _strided2(recips[0:128, 0:1], (1, 10), (0, 32))
                    dst_v = _strided2(outst[0:128, 320 * g:320 * g + 1],
                                      (32, 10), (1, 32))
                    nc.vector.tensor_mul(dst_v, num_v, rec_b)

                    # out-DMA per quad on the Pool queue, emitted at the
                    # start of the next bh iteration (or at the end)
                    def _emit_out(i=i, g=g, outst=outst):
                        nc.gpsimd.dma_start(
                            out=out_d[i, :, 320 * g:320 * g + 320],
                            in_=outst[:, 320 * g:320 * g + 320])
                    pending_outs.append(_emit_out)
            for fn in pending_outs:
                fn()
    nc.compile()
    return nc


_NC = None


def _get_nc():
    global _NC
    if _NC is None:
        _NC = build_nc()
    return _NC


# ---------------------------------------------------------------- entry point

def kernel(q, k, v, feats_per_t, window_len, act_size, img_feat_size):
    assert int(feats_per_t) == F and int(window_len) == W
    assert int(act_size) == 16 and int(img_feat_size) == 196
    q = np.asarray(q, np.float32)
    k = np.asarray(k, np.float32)
    v = np.asarray(v, np.float32)

    packed = _pack_all(q, k, v)
    in_maps = []
    for core in range(N_CORES):
        s = slice(BH_PER_CORE * core, BH_PER_CORE * (core + 1))
        in_maps.append({n: np.ascontiguousarray(a[s]) for n, a in packed.items()})

    nc = _get_nc()
    res = run_bass_kernel_spmd(nc, in_maps, list(range(N_CORES)))
    out = np.empty((B * H, S, D), np.float32)
    for core in range(N_CORES):
        out[BH_PER_CORE * core:BH_PER_CORE * (core + 1)] = _unpack(
            res.results[core]["out"])
    return out.reshape(B, H, S, D)
